# revision 16
# baseline (speedup 1.0000x reference)
"""Trainium2 Bass kernel for shifted-window correlation (27 shifts) + SE gate.

Reference computation (shapes hardcoded; B=1, C=16, W=80, H=96, D=112):
  corr[w,h,d,k] = mean_c x1[c,w,h,d] * x2[c, w+sx, h+sy, d+sz]   (zero-padded)
  s = mean_{w,h,d} corr;  g = sigmoid(w1 @ relu(w0 @ s + b0) + b1)
  out = corr * g

Strategy (8 cores, W sharded 10/core):
  - SBUF partition dim = (c:16, h8:8) where h8 = h // (H/8).
  - x2 loaded ONCE per parity (even/odd d for bf16 4B alignment) as a
    [128, HB+2, Wc+2, D(+2)] tile whose hblk axis carries a 1-row halo:
    row r holds h = h8*HB + (r-1), so all three sy shifts are free-dim
    offsets (the halo rows hold the neighboring h8 block's edge data).
  - Products on DVE (bf16 2x) with ~7 shifts/row offloaded to the idle
    Pool engine; channel reduction on the PE via a fixed block-diagonal
    selection matmul packing (k,h8) into 128/88-row PSUM tiles. PE does
    A-tile shifts then B-tile shifts per row so A drains overlap B
    matmuls; within each phase column-groups round-robin so weight loads
    overlap streaming.
  - corr stays resident in SBUF (no DRAM spill); ACT drains PSUM->SBUF
    capturing squeeze partials via accum_out.
  - Squeeze allreduce split: rows 0..HB-2 reduced early (latency hidden
    under the last row), last row folded into a second tiny allreduce.
  - Gated writeout straight from SBUF: A rows on ACT (per-partition
    scale), B rows on DVE (4x tensor_scalar), per-row output DMAs.
"""

import sys
import types

import numpy as np
import ml_dtypes


def _install_ntff_hook_shim():
    """agent image's antenv lacks axon_hooks; needed only for trace=True."""
    if "antenv.axon_hooks" in sys.modules:
        return
    try:
        import antenv
        from trn_agent_boot.trn_boot import _ntff_profile_via_ctypes

        hook = _ntff_profile_via_ctypes("/opt/axon/libaxon_pjrt.so")
        mod = types.ModuleType("antenv.axon_hooks")
        ref = {"h": hook}
        mod.get_axon_ntff_profile_hook = lambda: ref["h"]
        mod.set_axon_ntff_profile_hook = lambda h: ref.__setitem__("h", h)
        sys.modules["antenv.axon_hooks"] = mod
        antenv.axon_hooks = mod
    except Exception:
        pass


_install_ntff_hook_shim()

import concourse.bacc as bacc  # noqa: E402
import concourse.tile as tile  # noqa: E402
import concourse.mybir as mybir  # noqa: E402
from concourse.bass_utils import run_bass_kernel_spmd  # noqa: E402

BF16 = mybir.dt.bfloat16
FP32 = mybir.dt.float32
AF = mybir.ActivationFunctionType
ALU = mybir.AluOpType

N_CORES = 8
C = 16
H8 = 8          # partition sub-dim over h
K = 27
MID = 6

# shifts whose products run on the Pool engine (DVE handles the rest).
# Empty: Pool's software tensor_tensor is ~3us/row-product AND its SBUF
# reads contend with DVE, knocking DVE products out of 2x mode.
POOL_SHIFTS = frozenset()
POOL_STT = False  # walrus rejects scalar_tensor_tensor on Pool


class Cfg:
    def __init__(self, W=80, H=96, D=112):
        assert H % H8 == 0
        self.W, self.H, self.D = W, H, D
        self.Wc = W // N_CORES          # w columns per core
        self.HB = H // H8               # hblk extent (free dim)
        self.De = D + 2                 # odd-copy d extent
        self.FD = self.Wc * D           # flat (w, d) free size per row
        self.slices = [(o, min(o + 512, self.FD))
                       for o in range(0, self.FD, 512)]
        self.n_fs = len(self.slices)
        assert self.HB % 2 == 0 and self.HB >= 2
        self.groups = [(j, 1) for j in range(self.HB)]


# shift order matches reference: k = dx*9 + dy*3 + dz, s* = d*-1
SHIFTS = [(dx - 1, dy - 1, dz - 1)
          for dx in range(3) for dy in range(3) for dz in range(3)]

# PE consumption order: zip the tile-A chain (PSUM banks psA*) with the
# tile-B chain (banks psB*) so consecutive matmuls alternate banks and
# mostly alternate PE column groups, while each bank keeps a single open
# accumulation group at a time. B starts at group 1 to de-align positions.
_A_CHAIN = [4 * g + v for g in range(4) for v in range(4)]
_B_CHAIN = [16 + 4 * g + v for g in (1, 2, 0) for v in range(4 if g < 2 else 3)]
PE_ORDER = []
for _i in range(16):
    PE_ORDER.append(_A_CHAIN[_i])
    if _i < 11:
        PE_ORDER.append(_B_CHAIN[_i])


def _gv_of(k):
    """(is_A, psum column group, selection slice) for shift k."""
    kk = k if k < 16 else k - 16
    return k < 16, kk // 4, kk % 4


def _row_of(k, h8):
    """corr partition row for (k, h8). Tile A: k 0..15, tile B: 16..26."""
    kk = k if k < 16 else k - 16
    base = 0 if k < 16 else 128
    return base + 32 * (kk // 4) + 8 * (kk % 4) + h8


def build_nc(cfg: Cfg):
    nc = bacc.Bacc("TRN2", target_bir_lowering=False, debug=False,
                   num_devices=N_CORES)
    HB, Wc, D, De, FD = cfg.HB, cfg.Wc, cfg.D, cfg.De, cfg.FD

    x1_d = nc.dram_tensor("x1", [128, HB, Wc, D], BF16, kind="ExternalInput")
    x2e_d = nc.dram_tensor("x2e", [128, HB + 2, Wc + 2, D], BF16,
                           kind="ExternalInput")
    x2o_d = nc.dram_tensor("x2o", [128, HB + 2, Wc + 2, De], BF16,
                           kind="ExternalInput")
    sel_d = nc.dram_tensor("selmats", [128, 128], BF16, kind="ExternalInput")
    w0a_d = nc.dram_tensor("w0a", [128, MID], FP32, kind="ExternalInput")
    w0b_d = nc.dram_tensor("w0b", [88, MID], FP32, kind="ExternalInput")
    w1a_d = nc.dram_tensor("w1ra", [MID, 128], FP32, kind="ExternalInput")
    w1b_d = nc.dram_tensor("w1rb", [MID, 88], FP32, kind="ExternalInput")
    b0_d = nc.dram_tensor("b0c", [MID, 1], FP32, kind="ExternalInput")
    b1a_d = nc.dram_tensor("b1ra", [128, 1], FP32, kind="ExternalInput")
    b1b_d = nc.dram_tensor("b1rb", [88, 1], FP32, kind="ExternalInput")
    out_d = nc.dram_tensor("out", [216, HB, FD], BF16, kind="ExternalOutput")

    n_drain = HB * cfg.n_fs
    n_late = min(2, HB - 1)                 # rows covered by the final cc
    early_drains = (HB - n_late) * cfg.n_fs
    wh = (Wc + 2) // 2 or 1         # x2 w-half for split loads
    xh = max(Wc // 2, 1)            # x1 w-half

    with tile.TileContext(nc) as tc:
        with (
            tc.tile_pool(name="const", bufs=1) as cpool,
            tc.tile_pool(name="ps", bufs=1, space="PSUM") as ps,
            tc.tile_pool(name="dram", bufs=1, space="DRAM") as dram,
            tc.tile_pool(name="pp", bufs=5) as ppool,
            tc.tile_pool(name="qq", bufs=2) as qpool,
            tc.tile_pool(name="stage", bufs=2) as spool,
        ):
            # resident tiles
            x1t = cpool.tile([128, HB, Wc, D], BF16)
            x2e_t = cpool.tile([128, HB + 2, Wc + 2, D], BF16)
            x2o_t = cpool.tile([128, HB + 2, Wc + 2, De], BF16)
            corrA = cpool.tile([128, HB, FD], BF16)
            corrB = cpool.tile([88, HB, FD], BF16)
            selt = cpool.tile([128, 128], BF16)
            w0at = cpool.tile([128, MID], FP32)
            w0bt = cpool.tile([88, MID], FP32)
            w1at = cpool.tile([MID, 128], FP32)
            w1bt = cpool.tile([MID, 88], FP32)
            b0t = cpool.tile([MID, 1], FP32)
            b1at = cpool.tile([128, 1], FP32)
            b1bt = cpool.tile([88, 1], FP32)
            accA = cpool.tile([128, n_drain], FP32)
            accB = cpool.tile([88, n_drain], FP32)

            nc.sync.dma_start(selt[:], sel_d[:])
            nc.sync.dma_start(w0at[:], w0a_d[:])
            nc.sync.dma_start(w0bt[:], w0b_d[:])
            nc.sync.dma_start(w1at[:], w1a_d[:])
            nc.sync.dma_start(w1bt[:], w1b_d[:])
            nc.sync.dma_start(b0t[:], b0_d[:])
            nc.sync.dma_start(b1at[:], b1a_d[:])
            nc.sync.dma_start(b1bt[:], b1b_d[:])

            def load_x1_row(r):
                nc.sync.dma_start(x1t[:, r, 0:xh, :], x1_d[:, r, 0:xh, :])
                if xh < Wc:
                    nc.sync.dma_start(x1t[:, r, xh:Wc, :],
                                      x1_d[:, r, xh:Wc, :])

            def load_x2_row(r):
                nc.sync.dma_start(x2e_t[:, r, 0:wh, :], x2e_d[:, r, 0:wh, :])
                nc.sync.dma_start(x2e_t[:, r, wh:Wc + 2, :],
                                  x2e_d[:, r, wh:Wc + 2, :])
                nc.sync.dma_start(x2o_t[:, r, 0:wh, :], x2o_d[:, r, 0:wh, :])
                nc.sync.dma_start(x2o_t[:, r, wh:Wc + 2, :],
                                  x2o_d[:, r, wh:Wc + 2, :])

            # priority-ordered input loads: first rows first
            load_x1_row(0)
            if HB > 1:
                load_x1_row(1)
            for r in range(min(4, HB + 2)):
                load_x2_row(r)

            # Warm-up collective: absorbs cross-core launch skew and CC
            # firmware setup so the real allreduces only pay marginal latency.
            warm_in = dram.tile([MID, 1], FP32)
            warm_out = dram.tile([MID, 1], FP32)
            nc.sync.dma_start(warm_in[:], b0_d[:])
            nc.gpsimd.collective_compute(
                "AllReduce", ALU.add,
                replica_groups=[list(range(N_CORES))],
                ins=[warm_in[:].opt()],
                outs=[warm_out[:].opt()],
            )

            # remaining loads, interleaved in order of first use
            nx1 = 2
            for r in range(4, HB + 2, 2):
                while nx1 < min(r - 1, HB):
                    load_x1_row(nx1)
                    nx1 += 1
                load_x2_row(r)
                if r + 1 < HB + 2:
                    load_x2_row(r + 1)
            while nx1 < HB:
                load_x1_row(nx1)
                nx1 += 1

            cc1_in = dram.tile([216, 1], FP32)
            cc1_out = dram.tile([216, 1], FP32)
            cc_in = dram.tile([216, 1], FP32)
            cc_out = dram.tile([216, 1], FP32)

            a_tot = {g: 4 for g in range(4)}
            b_tot = {0: 4, 1: 4, 2: 3}

            def emit_products(j):
                """products for all 27 shifts of row j."""
                prods = {}
                pool_ks = [k for k in PE_ORDER if k in POOL_SHIFTS]
                dve_ks = [k for k in PE_ORDER if k not in POOL_SHIFTS]
                for k in pool_ks + dve_ks:
                    sx, sy, sz = SHIFTS[k]
                    if sz == 0:
                        src = x2e_t[:, 1 + j + sy, 1 + sx:1 + sx + Wc, 0:D]
                    else:
                        doff = sz + 1
                        src = x2o_t[:, 1 + j + sy, 1 + sx:1 + sx + Wc,
                                    doff:doff + D]
                    x1s = x1t[:, j, :, :]
                    if k in POOL_SHIFTS:
                        pt = qpool.tile([128, FD], BF16, tag="Q", bufs=3)
                        dst = pt.rearrange("p (w d) -> p w d", d=D)
                        if POOL_STT:
                            nc.gpsimd.scalar_tensor_tensor(
                                dst, x1s, 1.0, src, ALU.mult, ALU.mult)
                        else:
                            nc.gpsimd.tensor_tensor(dst, x1s, src, ALU.mult)
                    else:
                        pt = ppool.tile([128, FD], BF16, tag="P", bufs=5)
                        dst = pt.rearrange("p (w d) -> p w d", d=D)
                        nc.vector.tensor_tensor(dst, x1s, src, ALU.mult)
                    prods[k] = pt
                return prods

            def emit_reduce_row(j, prods):
                """PE reduction + ACT drains for row j."""
                psA = [ps.tile([128, hi - lo], FP32, tag=f"psA{i}",
                               name=f"psA{i}", padded_shape=[128, 512])
                       for i, (lo, hi) in enumerate(cfg.slices)]
                psB = [ps.tile([128, hi - lo], FP32, tag=f"psB{i}",
                               name=f"psB{i}", padded_shape=[128, 512])
                       for i, (lo, hi) in enumerate(cfg.slices)]
                seen = {}
                for k in PE_ORDER:
                    is_a, g, v = _gv_of(k)
                    tot = a_tot[g] if is_a else b_tot[g]
                    cnt = seen.get((is_a, g), 0)
                    seen[(is_a, g)] = cnt + 1
                    pst = psA if is_a else psB
                    for i, (lo, hi) in enumerate(cfg.slices):
                        nc.tensor.matmul(
                            pst[i][32 * g:32 * g + 32, :],
                            selt[:, 32 * v:32 * v + 32],
                            prods[k][:, lo:hi],
                            start=(cnt == 0), stop=(cnt == tot - 1),
                            tile_position=(0, 32 * g),
                        )
                    if k == _B_CHAIN[-1]:
                        # B chain done while A's tail streams: drain B now
                        for i, (lo, hi) in enumerate(cfg.slices):
                            di = j * cfg.n_fs + i
                            nc.scalar.activation(
                                corrB[:, j, lo:hi], psB[i][0:88, :], AF.Copy,
                                accum_out=accB[:, di:di + 1])
                for i, (lo, hi) in enumerate(cfg.slices):
                    di = j * cfg.n_fs + i
                    nc.scalar.activation(
                        corrA[:, j, lo:hi], psA[i][:], AF.Copy,
                        accum_out=accA[:, di:di + 1])

            for gi, (j0, nr) in enumerate(cfg.groups):
                prods = emit_products(j0)
                if gi == HB - n_late:
                    # early allreduce over rows 0..HB-n_late-1 partials; its
                    # latency hides under the last rows' compute.
                    scrA1 = cpool.tile([128, early_drains], FP32)
                    scrB1 = cpool.tile([88, early_drains], FP32)
                    pA1 = cpool.tile([128, 1], FP32)
                    pB1 = cpool.tile([88, 1], FP32)
                    nc.scalar.activation(scrA1[:], accA[:, 0:early_drains],
                                         AF.Copy, accum_out=pA1[:])
                    nc.scalar.activation(scrB1[:], accB[:, 0:early_drains],
                                         AF.Copy, accum_out=pB1[:])
                    nc.sync.dma_start(cc1_in[0:128, :], pA1[:])
                    nc.sync.dma_start(cc1_in[128:216, :], pB1[:])
                    nc.gpsimd.collective_compute(
                        "AllReduce", ALU.add,
                        replica_groups=[list(range(N_CORES))],
                        ins=[cc1_in[:].opt()],
                        outs=[cc1_out[:].opt()],
                    )
                emit_reduce_row(j0, prods)

            # ---- last-rows partials + final allreduce + gate MLP ----
            # cc2 carries the raw last-rows partials; the MLP's first layer
            # is linear in s, so cc1's and cc2's contributions accumulate as
            # four matmuls into one PSUM tile (no fold / extra DMA hop).
            pA = cpool.tile([128, 1], FP32)
            pB = cpool.tile([88, 1], FP32)
            scrA = cpool.tile([128, n_drain - early_drains], FP32)
            scrB = cpool.tile([88, n_drain - early_drains], FP32)
            nc.scalar.activation(scrA[:], accA[:, early_drains:n_drain],
                                 AF.Copy, accum_out=pA[:])
            nc.scalar.activation(scrB[:], accB[:, early_drains:n_drain],
                                 AF.Copy, accum_out=pB[:])
            nc.sync.dma_start(cc_in[0:128, :], pA[:])
            nc.sync.dma_start(cc_in[128:216, :], pB[:])
            nc.gpsimd.collective_compute(
                "AllReduce", ALU.add,
                replica_groups=[list(range(N_CORES))],
                ins=[cc_in[:].opt()],
                outs=[cc_out[:].opt()],
            )
            pAg1 = cpool.tile([128, 1], FP32)
            pBg1 = cpool.tile([88, 1], FP32)
            nc.sync.dma_start(pAg1[:], cc1_out[0:128, :])
            nc.sync.dma_start(pBg1[:], cc1_out[128:216, :])
            pAg = cpool.tile([128, 1], FP32)
            pBg = cpool.tile([88, 1], FP32)
            nc.sync.dma_start(pAg[:], cc_out[0:128, :])
            nc.sync.dma_start(pBg[:], cc_out[128:216, :])

            hps = ps.tile([MID, 1], FP32, tag="psA0", padded_shape=[128, 512])
            nc.tensor.matmul(hps[:], w0at[:], pAg1[:], start=True, stop=False)
            nc.tensor.matmul(hps[:], w0bt[:], pBg1[:], start=False, stop=False)
            nc.tensor.matmul(hps[:], w0at[:], pAg[:], start=False, stop=False)
            nc.tensor.matmul(hps[:], w0bt[:], pBg[:], start=False, stop=True)
            hvec = cpool.tile([MID, 1], FP32)
            nc.scalar.activation(hvec[:], hps[:], AF.Relu, bias=b0t[:],
                                 scale=1.0)
            gpsA = ps.tile([128, 1], FP32, tag="psA1", padded_shape=[128, 512])
            gpsB = ps.tile([88, 1], FP32, tag="psA2", padded_shape=[128, 512])
            nc.tensor.matmul(gpsA[:], w1at[:], hvec[:], start=True, stop=True)
            nc.tensor.matmul(gpsB[:], w1bt[:], hvec[:], start=True, stop=True)
            gA = cpool.tile([128, 1], FP32)
            gB = cpool.tile([88, 1], FP32)
            nc.scalar.activation(gA[:], gpsA[:], AF.Sigmoid, bias=b1at[:],
                                 scale=1.0)
            nc.scalar.activation(gB[:], gpsB[:], AF.Sigmoid, bias=b1bt[:],
                                 scale=1.0)

            # ---- gated writeout from SBUF (A on ACT, B on DVE 4x).
            # Half-row output DMAs, all issued from the idle Pool sequencer
            # (cheapest DGE dispatch) to keep ACT/SP free for gating. ----
            fh = (FD // 2 + 1) & ~1 if FD > 2 else FD
            for j in range(HB):
                stA = spool.tile([128, FD], BF16, tag="gsA", bufs=3)
                nc.scalar.mul(stA[:], corrA[:, j, :], gA[:])
                stB = spool.tile([88, FD], BF16, tag="gsB", bufs=3)
                nc.vector.tensor_scalar(stB[:], corrB[:, j, :], gB[:],
                                        None, ALU.mult)
                nc.gpsimd.dma_start(out_d[0:128, j, 0:fh], stA[:, 0:fh])
                nc.gpsimd.dma_start(out_d[0:128, j, fh:FD], stA[:, fh:FD])
                nc.gpsimd.dma_start(out_d[128:216, j, 0:fh], stB[:, 0:fh])
                nc.gpsimd.dma_start(out_d[128:216, j, fh:FD], stB[:, fh:FD])

    nc.compile()
    return nc


# ---------------- host-side prep / assembly ----------------

def make_gate_consts(w0, b0, w1, b1, cfg: Cfg):
    norm = 1.0 / (cfg.W * cfg.H * cfg.D)
    sel = np.zeros((128, 128), dtype=np.float32)
    for v in range(4):
        for c in range(C):
            for h8 in range(H8):
                sel[c * H8 + h8, 32 * v + 8 * v + h8] = 1.0 / 16
    w0 = np.asarray(w0, dtype=np.float32)
    w1 = np.asarray(w1, dtype=np.float32)
    b1 = np.asarray(b1, dtype=np.float32)
    w0a = np.zeros((128, MID), dtype=np.float32)
    w0b = np.zeros((88, MID), dtype=np.float32)
    w1ra = np.zeros((MID, 128), dtype=np.float32)
    w1rb = np.zeros((MID, 88), dtype=np.float32)
    b1ra = np.zeros((128, 1), dtype=np.float32)
    b1rb = np.zeros((88, 1), dtype=np.float32)
    for k in range(K):
        for h8 in range(H8):
            r = _row_of(k, h8)
            if k < 16:
                w0a[r, :] = w0[:, k] * norm
                w1ra[:, r] = w1[k, :]
                b1ra[r, 0] = b1[k]
            else:
                w0b[r - 128, :] = w0[:, k] * norm
                w1rb[:, r - 128] = w1[k, :]
                b1rb[r - 128, 0] = b1[k]
    return {
        "selmats": sel.astype(ml_dtypes.bfloat16),
        "w0a": w0a, "w0b": w0b, "w1ra": w1ra, "w1rb": w1rb,
        "b0c": np.asarray(b0, dtype=np.float32).reshape(MID, 1),
        "b1ra": b1ra, "b1rb": b1rb,
    }


def _fold(a, HB):
    # [C, w, H, D'] -> [(c h8), hblk, w, d]
    Cc, ww, hh, dd = a.shape
    a = a.reshape(Cc, ww, H8, HB, dd)
    a = np.ascontiguousarray(a.transpose(0, 2, 3, 1, 4))
    return a.reshape(C * H8, HB, ww, dd)


def make_inputs_per_core(x_1, x_2, w0, b0, w1, b1, cfg: Cfg):
    """x_1/x_2: [1, C, W, H, D] float32 -> list of per-core input dicts."""
    W, H, D, De = cfg.W, cfg.H, cfg.D, cfg.De
    Wc, HB = cfg.Wc, cfg.HB
    x1 = np.asarray(x_1)[0].astype(ml_dtypes.bfloat16)      # [C, W, H, D]
    x2 = np.asarray(x_2)[0].astype(ml_dtypes.bfloat16)
    # padded x2: w +-1, h +-1, d in [-1, D+1)
    x2p = np.zeros((C, W + 2, H + 2, D + 2), dtype=ml_dtypes.bfloat16)
    x2p[:, 1:W + 1, 1:H + 1, 1:D + 1] = x2
    # hblk-extended h indices: row r of (h8) block = x2p h-index h8*HB + r,
    # covering h = h8*HB - 1 .. (h8+1)*HB (1-voxel halo on both sides)
    hidx = (np.arange(H8) * HB)[:, None] + np.arange(HB + 2)  # [H8, HB+2]

    consts = make_gate_consts(w0, b0, w1, b1, cfg)
    in_maps = []
    for ci in range(N_CORES):
        ws = ci * Wc
        m = dict(consts)
        m["x1"] = _fold(x1[:, ws:ws + Wc, :, :], HB)
        blk = x2p[:, ws:ws + Wc + 2, :, :]                  # [C, Wc+2, H+2, De]
        ee = blk[:, :, hidx, 1:1 + D]                       # [C, Wc+2, H8, HB+2, D]
        oo = blk[:, :, hidx, 0:De]
        m["x2e"] = np.ascontiguousarray(
            ee.transpose(0, 2, 3, 1, 4)).reshape(128, HB + 2, Wc + 2, D)
        m["x2o"] = np.ascontiguousarray(
            oo.transpose(0, 2, 3, 1, 4)).reshape(128, HB + 2, Wc + 2, De)
        in_maps.append(m)
    return in_maps


def assemble_output(results, cfg: Cfg):
    W, H, D = cfg.W, cfg.H, cfg.D
    Wc, HB = cfg.Wc, cfg.HB
    rows = np.empty((K, H8), dtype=np.int64)
    for k in range(K):
        for h8 in range(H8):
            rows[k, h8] = _row_of(k, h8)
    out = np.empty((W, H, D, K), dtype=np.float32)
    for ci, r in enumerate(results):
        o = np.asarray(r["out"]).reshape(216, HB, Wc, D)
        core = o[rows]                        # [K, H8, HB, Wc, D]
        core = core.transpose(3, 1, 2, 4, 0)  # [Wc, H8, HB, D, K]
        out[ci * Wc:(ci + 1) * Wc] = core.reshape(Wc, H, D, K)
    return out[None]


_CACHE = {}
TRACE = False           # test harness can set kernel.TRACE = True


def kernel(x_1, x_2, w0, b0, w1, b1):
    cfg = Cfg()
    if "nc" not in _CACHE:
        _CACHE["nc"] = build_nc(cfg)
    nc = _CACHE["nc"]
    in_maps = make_inputs_per_core(x_1, x_2, w0, b0, w1, b1, cfg)
    last_exc = None
    for _attempt in range(3):
        try:
            res = run_bass_kernel_spmd(nc, in_maps,
                                       core_ids=list(range(N_CORES)),
                                       trace=TRACE)
            break
        except Exception as e:  # transient NRT device errors: retry
            last_exc = e
    else:
        raise last_exc
    _CACHE["last_res"] = res
    return assemble_output(res.results, cfg)


# revision 18
# speedup vs baseline: 1.0951x; 1.0951x over previous
"""Trainium2 Bass kernel for shifted-window correlation (27 shifts) + SE gate.

Reference computation (shapes hardcoded; B=1, C=16, W=80, H=96, D=112):
  corr[w,h,d,k] = mean_c x1[c,w,h,d] * x2[c, w+sx, h+sy, d+sz]   (zero-padded)
  s = mean_{w,h,d} corr;  g = sigmoid(w1 @ relu(w0 @ s + b0) + b1)
  out = corr * g

Strategy (8 cores, W sharded 10/core):
  - SBUF partition dim = (c:16, h8:8) where h8 = h // (H/8).
  - x2 loaded ONCE per parity (even/odd d for bf16 4B alignment) as a
    [128, HB+2, Wc+2, D(+2)] tile whose hblk axis carries a 1-row halo:
    row r holds h = h8*HB + (r-1), so all three sy shifts are free-dim
    offsets (the halo rows hold the neighboring h8 block's edge data).
  - Products on DVE (bf16 2x) with ~7 shifts/row offloaded to the idle
    Pool engine; channel reduction on the PE via a fixed block-diagonal
    selection matmul packing (k,h8) into 128/88-row PSUM tiles. PE does
    A-tile shifts then B-tile shifts per row so A drains overlap B
    matmuls; within each phase column-groups round-robin so weight loads
    overlap streaming.
  - corr stays resident in SBUF (no DRAM spill); ACT drains PSUM->SBUF
    capturing squeeze partials via accum_out.
  - Squeeze allreduce split: rows 0..HB-2 reduced early (latency hidden
    under the last row), last row folded into a second tiny allreduce.
  - Gated writeout straight from SBUF: A rows on ACT (per-partition
    scale), B rows on DVE (4x tensor_scalar), per-row output DMAs.
"""

import sys
import types

import numpy as np
import ml_dtypes


def _install_ntff_hook_shim():
    """agent image's antenv lacks axon_hooks; needed only for trace=True."""
    if "antenv.axon_hooks" in sys.modules:
        return
    try:
        import antenv
        from trn_agent_boot.trn_boot import _ntff_profile_via_ctypes

        hook = _ntff_profile_via_ctypes("/opt/axon/libaxon_pjrt.so")
        mod = types.ModuleType("antenv.axon_hooks")
        ref = {"h": hook}
        mod.get_axon_ntff_profile_hook = lambda: ref["h"]
        mod.set_axon_ntff_profile_hook = lambda h: ref.__setitem__("h", h)
        sys.modules["antenv.axon_hooks"] = mod
        antenv.axon_hooks = mod
    except Exception:
        pass


_install_ntff_hook_shim()

import concourse.bacc as bacc  # noqa: E402
import concourse.tile as tile  # noqa: E402
import concourse.mybir as mybir  # noqa: E402
from concourse.bass_utils import run_bass_kernel_spmd  # noqa: E402

BF16 = mybir.dt.bfloat16
FP32 = mybir.dt.float32
AF = mybir.ActivationFunctionType
ALU = mybir.AluOpType

N_CORES = 8
C = 16
H8 = 8          # partition sub-dim over h
K = 27
MID = 6

# shifts whose products run on the Pool engine (DVE handles the rest).
# Empty: Pool's software tensor_tensor is ~3us/row-product AND its SBUF
# reads contend with DVE, knocking DVE products out of 2x mode.
POOL_SHIFTS = frozenset()
POOL_STT = False  # walrus rejects scalar_tensor_tensor on Pool


class Cfg:
    def __init__(self, W=80, H=96, D=112):
        assert H % H8 == 0
        self.W, self.H, self.D = W, H, D
        self.Wc = W // N_CORES          # w columns per core
        self.HB = H // H8               # hblk extent (free dim)
        self.De = D + 2                 # odd-copy d extent
        self.FD = self.Wc * D           # flat (w, d) free size per row
        self.slices = [(o, min(o + 512, self.FD))
                       for o in range(0, self.FD, 512)]
        self.n_fs = len(self.slices)
        assert self.HB % 2 == 0 and self.HB >= 2
        self.groups = [(j, 1) for j in range(self.HB)]


# shift order matches reference: k = dx*9 + dy*3 + dz, s* = d*-1
SHIFTS = [(dx - 1, dy - 1, dz - 1)
          for dx in range(3) for dy in range(3) for dz in range(3)]

# PE consumption order: zip the tile-A chain (PSUM banks psA*) with the
# tile-B chain (banks psB*) so consecutive matmuls alternate banks and
# mostly alternate PE column groups, while each bank keeps a single open
# accumulation group at a time. B starts at group 1 to de-align positions.
_A_CHAIN = [4 * g + v for g in range(4) for v in range(4)]
_B_CHAIN = [16 + 4 * g + v for g in (1, 2, 0) for v in range(4 if g < 2 else 3)]
PE_ORDER = []
for _i in range(16):
    PE_ORDER.append(_A_CHAIN[_i])
    if _i < 11:
        PE_ORDER.append(_B_CHAIN[_i])


def _gv_of(k):
    """(is_A, psum column group, selection slice) for shift k."""
    kk = k if k < 16 else k - 16
    return k < 16, kk // 4, kk % 4


def _row_of(k, h8):
    """corr partition row for (k, h8). Tile A: k 0..15, tile B: 16..26."""
    kk = k if k < 16 else k - 16
    base = 0 if k < 16 else 128
    return base + 32 * (kk // 4) + 8 * (kk % 4) + h8


def build_nc(cfg: Cfg):
    nc = bacc.Bacc("TRN2", target_bir_lowering=False, debug=False,
                   num_devices=N_CORES)
    HB, Wc, D, De, FD = cfg.HB, cfg.Wc, cfg.D, cfg.De, cfg.FD

    x1_d = nc.dram_tensor("x1", [128, HB, Wc, D], BF16, kind="ExternalInput")
    x2e_d = nc.dram_tensor("x2e", [128, HB + 2, Wc + 2, D], BF16,
                           kind="ExternalInput")
    x2o_d = nc.dram_tensor("x2o", [128, HB + 2, Wc + 2, De], BF16,
                           kind="ExternalInput")
    sel_d = nc.dram_tensor("selmats", [128, 128], BF16, kind="ExternalInput")
    w0a_d = nc.dram_tensor("w0a", [128, MID], FP32, kind="ExternalInput")
    w0b_d = nc.dram_tensor("w0b", [88, MID], FP32, kind="ExternalInput")
    w1a_d = nc.dram_tensor("w1ra", [MID, 128], FP32, kind="ExternalInput")
    w1b_d = nc.dram_tensor("w1rb", [MID, 88], FP32, kind="ExternalInput")
    b0_d = nc.dram_tensor("b0c", [MID, 1], FP32, kind="ExternalInput")
    b1a_d = nc.dram_tensor("b1ra", [128, 1], FP32, kind="ExternalInput")
    b1b_d = nc.dram_tensor("b1rb", [88, 1], FP32, kind="ExternalInput")
    out_d = nc.dram_tensor("out", [216, HB, FD], BF16, kind="ExternalOutput")

    n_drain = HB * cfg.n_fs
    wh = (Wc + 2) // 2 or 1         # x2 w-half for split loads
    xh = max(Wc // 2, 1)            # x1 w-half

    with tile.TileContext(nc) as tc:
        with (
            tc.tile_pool(name="const", bufs=1) as cpool,
            tc.tile_pool(name="ps", bufs=1, space="PSUM") as ps,
            tc.tile_pool(name="dram", bufs=1, space="DRAM") as dram,
            tc.tile_pool(name="pp", bufs=5) as ppool,
            tc.tile_pool(name="qq", bufs=2) as qpool,
            tc.tile_pool(name="stage", bufs=2) as spool,
        ):
            # resident tiles
            x1t = cpool.tile([128, HB, Wc, D], BF16)
            x2e_t = cpool.tile([128, HB + 2, Wc + 2, D], BF16)
            x2o_t = cpool.tile([128, HB + 2, Wc + 2, De], BF16)
            corrA = cpool.tile([128, HB, FD], BF16)
            corrB = cpool.tile([88, HB, FD], BF16)
            selt = cpool.tile([128, 128], BF16)
            w0at = cpool.tile([128, MID], FP32)
            w0bt = cpool.tile([88, MID], FP32)
            w1at = cpool.tile([MID, 128], FP32)
            w1bt = cpool.tile([MID, 88], FP32)
            b0t = cpool.tile([MID, 1], FP32)
            b1at = cpool.tile([128, 1], FP32)
            b1bt = cpool.tile([88, 1], FP32)
            accA = cpool.tile([128, n_drain], FP32)
            accB = cpool.tile([88, n_drain], FP32)

            nc.sync.dma_start(selt[:], sel_d[:])
            nc.sync.dma_start(w0at[:], w0a_d[:])
            nc.sync.dma_start(w0bt[:], w0b_d[:])
            nc.sync.dma_start(w1at[:], w1a_d[:])
            nc.sync.dma_start(w1bt[:], w1b_d[:])
            nc.sync.dma_start(b0t[:], b0_d[:])
            nc.sync.dma_start(b1at[:], b1a_d[:])
            nc.sync.dma_start(b1bt[:], b1b_d[:])

            def load_x1_row(r):
                nc.sync.dma_start(x1t[:, r, 0:xh, :], x1_d[:, r, 0:xh, :])
                if xh < Wc:
                    nc.sync.dma_start(x1t[:, r, xh:Wc, :],
                                      x1_d[:, r, xh:Wc, :])

            def load_x2_row(r):
                nc.sync.dma_start(x2e_t[:, r, 0:wh, :], x2e_d[:, r, 0:wh, :])
                nc.sync.dma_start(x2e_t[:, r, wh:Wc + 2, :],
                                  x2e_d[:, r, wh:Wc + 2, :])
                nc.sync.dma_start(x2o_t[:, r, 0:wh, :], x2o_d[:, r, 0:wh, :])
                nc.sync.dma_start(x2o_t[:, r, wh:Wc + 2, :],
                                  x2o_d[:, r, wh:Wc + 2, :])

            # priority-ordered input loads: first rows first
            load_x1_row(0)
            if HB > 1:
                load_x1_row(1)
            for r in range(min(4, HB + 2)):
                load_x2_row(r)

            # Warm-up collective: absorbs cross-core launch skew and CC
            # firmware setup so the real allreduces only pay marginal latency.
            warm_in = dram.tile([MID, 1], FP32)
            warm_out = dram.tile([MID, 1], FP32)
            nc.sync.dma_start(warm_in[:], b0_d[:])
            nc.gpsimd.collective_compute(
                "AllReduce", ALU.add,
                replica_groups=[list(range(N_CORES))],
                ins=[warm_in[:].opt()],
                outs=[warm_out[:].opt()],
            )

            # remaining loads, interleaved in order of first use
            nx1 = 2
            for r in range(4, HB + 2, 2):
                while nx1 < min(r - 1, HB):
                    load_x1_row(nx1)
                    nx1 += 1
                load_x2_row(r)
                if r + 1 < HB + 2:
                    load_x2_row(r + 1)
            while nx1 < HB:
                load_x1_row(nx1)
                nx1 += 1

            cc_in = dram.tile([216, 1], FP32)
            cc_out = dram.tile([216, 1], FP32)

            a_tot = {g: 4 for g in range(4)}
            b_tot = {0: 4, 1: 4, 2: 3}

            def emit_products(j):
                """products for all 27 shifts of row j."""
                prods = {}
                pool_ks = [k for k in PE_ORDER if k in POOL_SHIFTS]
                dve_ks = [k for k in PE_ORDER if k not in POOL_SHIFTS]
                for k in pool_ks + dve_ks:
                    sx, sy, sz = SHIFTS[k]
                    if sz == 0:
                        src = x2e_t[:, 1 + j + sy, 1 + sx:1 + sx + Wc, 0:D]
                    else:
                        doff = sz + 1
                        src = x2o_t[:, 1 + j + sy, 1 + sx:1 + sx + Wc,
                                    doff:doff + D]
                    x1s = x1t[:, j, :, :]
                    if k in POOL_SHIFTS:
                        pt = qpool.tile([128, FD], BF16, tag="Q", bufs=3)
                        dst = pt.rearrange("p (w d) -> p w d", d=D)
                        if POOL_STT:
                            nc.gpsimd.scalar_tensor_tensor(
                                dst, x1s, 1.0, src, ALU.mult, ALU.mult)
                        else:
                            nc.gpsimd.tensor_tensor(dst, x1s, src, ALU.mult)
                    else:
                        pt = ppool.tile([128, FD], BF16, tag="P", bufs=5)
                        dst = pt.rearrange("p (w d) -> p w d", d=D)
                        nc.vector.tensor_tensor(dst, x1s, src, ALU.mult)
                    prods[k] = pt
                return prods

            def emit_reduce_row(j, prods):
                """PE reduction + ACT drains for row j."""
                psA = [ps.tile([128, hi - lo], FP32, tag=f"psA{i}",
                               name=f"psA{i}", padded_shape=[128, 512])
                       for i, (lo, hi) in enumerate(cfg.slices)]
                psB = [ps.tile([128, hi - lo], FP32, tag=f"psB{i}",
                               name=f"psB{i}", padded_shape=[128, 512])
                       for i, (lo, hi) in enumerate(cfg.slices)]
                seen = {}
                for k in PE_ORDER:
                    is_a, g, v = _gv_of(k)
                    tot = a_tot[g] if is_a else b_tot[g]
                    cnt = seen.get((is_a, g), 0)
                    seen[(is_a, g)] = cnt + 1
                    pst = psA if is_a else psB
                    for i, (lo, hi) in enumerate(cfg.slices):
                        nc.tensor.matmul(
                            pst[i][32 * g:32 * g + 32, :],
                            selt[:, 32 * v:32 * v + 32],
                            prods[k][:, lo:hi],
                            start=(cnt == 0), stop=(cnt == tot - 1),
                            tile_position=(0, 32 * g),
                        )
                    if k == _B_CHAIN[-1]:
                        # B chain done while A's tail streams: drain B now
                        for i, (lo, hi) in enumerate(cfg.slices):
                            di = j * cfg.n_fs + i
                            nc.scalar.activation(
                                corrB[:, j, lo:hi], psB[i][0:88, :], AF.Copy,
                                accum_out=accB[:, di:di + 1])
                for i, (lo, hi) in enumerate(cfg.slices):
                    di = j * cfg.n_fs + i
                    nc.scalar.activation(
                        corrA[:, j, lo:hi], psA[i][:], AF.Copy,
                        accum_out=accA[:, di:di + 1])

            for gi, (j0, nr) in enumerate(cfg.groups):
                prods = emit_products(j0)
                emit_reduce_row(j0, prods)

            # ---- squeeze partials + single allreduce + gate MLP ----
            pA = cpool.tile([128, 1], FP32)
            pB = cpool.tile([88, 1], FP32)
            scrA = cpool.tile([128, n_drain], FP32)
            scrB = cpool.tile([88, n_drain], FP32)
            nc.scalar.activation(scrA[:], accA[:, 0:n_drain],
                                 AF.Copy, accum_out=pA[:])
            nc.scalar.activation(scrB[:], accB[:, 0:n_drain],
                                 AF.Copy, accum_out=pB[:])
            nc.sync.dma_start(cc_in[0:128, :], pA[:])
            nc.sync.dma_start(cc_in[128:216, :], pB[:])
            nc.gpsimd.collective_compute(
                "AllReduce", ALU.add,
                replica_groups=[list(range(N_CORES))],
                ins=[cc_in[:].opt()],
                outs=[cc_out[:].opt()],
            )
            pAg = cpool.tile([128, 1], FP32)
            pBg = cpool.tile([88, 1], FP32)
            nc.sync.dma_start(pAg[:], cc_out[0:128, :])
            nc.sync.dma_start(pBg[:], cc_out[128:216, :])

            hps = ps.tile([MID, 1], FP32, tag="psA0", padded_shape=[128, 512])
            nc.tensor.matmul(hps[:], w0at[:], pAg[:], start=True, stop=False)
            nc.tensor.matmul(hps[:], w0bt[:], pBg[:], start=False, stop=True)
            hvec = cpool.tile([MID, 1], FP32)
            nc.scalar.activation(hvec[:], hps[:], AF.Relu, bias=b0t[:],
                                 scale=1.0)
            gpsA = ps.tile([128, 1], FP32, tag="psA1", padded_shape=[128, 512])
            gpsB = ps.tile([88, 1], FP32, tag="psA2", padded_shape=[128, 512])
            nc.tensor.matmul(gpsA[:], w1at[:], hvec[:], start=True, stop=True)
            nc.tensor.matmul(gpsB[:], w1bt[:], hvec[:], start=True, stop=True)
            gA = cpool.tile([128, 1], FP32)
            gB = cpool.tile([88, 1], FP32)
            nc.scalar.activation(gA[:], gpsA[:], AF.Sigmoid, bias=b1at[:],
                                 scale=1.0)
            nc.scalar.activation(gB[:], gpsB[:], AF.Sigmoid, bias=b1bt[:],
                                 scale=1.0)

            # ---- gated writeout from SBUF (A on ACT, B on DVE 4x).
            # Half-row output DMAs, all issued from the idle Pool sequencer
            # (cheapest DGE dispatch) to keep ACT/SP free for gating. ----
            fh = (FD // 2 + 1) & ~1 if FD > 2 else FD
            for j in range(HB):
                stA = spool.tile([128, FD], BF16, tag="gsA", bufs=4)
                nc.scalar.mul(stA[:], corrA[:, j, :], gA[:])
                stB = spool.tile([88, FD], BF16, tag="gsB", bufs=4)
                nc.vector.tensor_scalar(stB[:], corrB[:, j, :], gB[:],
                                        None, ALU.mult)
                nc.gpsimd.dma_start(out_d[0:128, j, 0:fh], stA[:, 0:fh])
                nc.gpsimd.dma_start(out_d[0:128, j, fh:FD], stA[:, fh:FD])
                nc.gpsimd.dma_start(out_d[128:216, j, 0:fh], stB[:, 0:fh])
                nc.gpsimd.dma_start(out_d[128:216, j, fh:FD], stB[:, fh:FD])

    nc.compile()
    return nc


# ---------------- host-side prep / assembly ----------------

def make_gate_consts(w0, b0, w1, b1, cfg: Cfg):
    norm = 1.0 / (cfg.W * cfg.H * cfg.D)
    sel = np.zeros((128, 128), dtype=np.float32)
    for v in range(4):
        for c in range(C):
            for h8 in range(H8):
                sel[c * H8 + h8, 32 * v + 8 * v + h8] = 1.0 / 16
    w0 = np.asarray(w0, dtype=np.float32)
    w1 = np.asarray(w1, dtype=np.float32)
    b1 = np.asarray(b1, dtype=np.float32)
    w0a = np.zeros((128, MID), dtype=np.float32)
    w0b = np.zeros((88, MID), dtype=np.float32)
    w1ra = np.zeros((MID, 128), dtype=np.float32)
    w1rb = np.zeros((MID, 88), dtype=np.float32)
    b1ra = np.zeros((128, 1), dtype=np.float32)
    b1rb = np.zeros((88, 1), dtype=np.float32)
    for k in range(K):
        for h8 in range(H8):
            r = _row_of(k, h8)
            if k < 16:
                w0a[r, :] = w0[:, k] * norm
                w1ra[:, r] = w1[k, :]
                b1ra[r, 0] = b1[k]
            else:
                w0b[r - 128, :] = w0[:, k] * norm
                w1rb[:, r - 128] = w1[k, :]
                b1rb[r - 128, 0] = b1[k]
    return {
        "selmats": sel.astype(ml_dtypes.bfloat16),
        "w0a": w0a, "w0b": w0b, "w1ra": w1ra, "w1rb": w1rb,
        "b0c": np.asarray(b0, dtype=np.float32).reshape(MID, 1),
        "b1ra": b1ra, "b1rb": b1rb,
    }


def _fold(a, HB):
    # [C, w, H, D'] -> [(c h8), hblk, w, d]
    Cc, ww, hh, dd = a.shape
    a = a.reshape(Cc, ww, H8, HB, dd)
    a = np.ascontiguousarray(a.transpose(0, 2, 3, 1, 4))
    return a.reshape(C * H8, HB, ww, dd)


def make_inputs_per_core(x_1, x_2, w0, b0, w1, b1, cfg: Cfg):
    """x_1/x_2: [1, C, W, H, D] float32 -> list of per-core input dicts."""
    W, H, D, De = cfg.W, cfg.H, cfg.D, cfg.De
    Wc, HB = cfg.Wc, cfg.HB
    x1 = np.asarray(x_1)[0].astype(ml_dtypes.bfloat16)      # [C, W, H, D]
    x2 = np.asarray(x_2)[0].astype(ml_dtypes.bfloat16)
    # padded x2: w +-1, h +-1, d in [-1, D+1)
    x2p = np.zeros((C, W + 2, H + 2, D + 2), dtype=ml_dtypes.bfloat16)
    x2p[:, 1:W + 1, 1:H + 1, 1:D + 1] = x2
    # hblk-extended h indices: row r of (h8) block = x2p h-index h8*HB + r,
    # covering h = h8*HB - 1 .. (h8+1)*HB (1-voxel halo on both sides)
    hidx = (np.arange(H8) * HB)[:, None] + np.arange(HB + 2)  # [H8, HB+2]

    consts = make_gate_consts(w0, b0, w1, b1, cfg)
    in_maps = []
    for ci in range(N_CORES):
        ws = ci * Wc
        m = dict(consts)
        m["x1"] = _fold(x1[:, ws:ws + Wc, :, :], HB)
        blk = x2p[:, ws:ws + Wc + 2, :, :]                  # [C, Wc+2, H+2, De]
        ee = blk[:, :, hidx, 1:1 + D]                       # [C, Wc+2, H8, HB+2, D]
        oo = blk[:, :, hidx, 0:De]
        m["x2e"] = np.ascontiguousarray(
            ee.transpose(0, 2, 3, 1, 4)).reshape(128, HB + 2, Wc + 2, D)
        m["x2o"] = np.ascontiguousarray(
            oo.transpose(0, 2, 3, 1, 4)).reshape(128, HB + 2, Wc + 2, De)
        in_maps.append(m)
    return in_maps


def assemble_output(results, cfg: Cfg):
    W, H, D = cfg.W, cfg.H, cfg.D
    Wc, HB = cfg.Wc, cfg.HB
    rows = np.empty((K, H8), dtype=np.int64)
    for k in range(K):
        for h8 in range(H8):
            rows[k, h8] = _row_of(k, h8)
    out = np.empty((W, H, D, K), dtype=np.float32)
    for ci, r in enumerate(results):
        o = np.asarray(r["out"]).reshape(216, HB, Wc, D)
        core = o[rows]                        # [K, H8, HB, Wc, D]
        core = core.transpose(3, 1, 2, 4, 0)  # [Wc, H8, HB, D, K]
        out[ci * Wc:(ci + 1) * Wc] = core.reshape(Wc, H, D, K)
    return out[None]


_CACHE = {}
TRACE = False           # test harness can set kernel.TRACE = True


def kernel(x_1, x_2, w0, b0, w1, b1):
    cfg = Cfg()
    if "nc" not in _CACHE:
        _CACHE["nc"] = build_nc(cfg)
    nc = _CACHE["nc"]
    in_maps = make_inputs_per_core(x_1, x_2, w0, b0, w1, b1, cfg)
    last_exc = None
    for _attempt in range(3):
        try:
            res = run_bass_kernel_spmd(nc, in_maps,
                                       core_ids=list(range(N_CORES)),
                                       trace=TRACE)
            break
        except Exception as e:  # transient NRT device errors: retry
            last_exc = e
    else:
        raise last_exc
    _CACHE["last_res"] = res
    return assemble_output(res.results, cfg)


# revision 19
# speedup vs baseline: 1.1056x; 1.0096x over previous
"""Trainium2 Bass kernel for shifted-window correlation (27 shifts) + SE gate.

Reference computation (shapes hardcoded; B=1, C=16, W=80, H=96, D=112):
  corr[w,h,d,k] = mean_c x1[c,w,h,d] * x2[c, w+sx, h+sy, d+sz]   (zero-padded)
  s = mean_{w,h,d} corr;  g = sigmoid(w1 @ relu(w0 @ s + b0) + b1)
  out = corr * g

Strategy (8 cores, W sharded 10/core):
  - SBUF partition dim = (c:16, h8:8) where h8 = h // (H/8).
  - x2 loaded ONCE per parity (even/odd d for bf16 4B alignment) as a
    [128, HB+2, Wc+2, D(+2)] tile whose hblk axis carries a 1-row halo:
    row r holds h = h8*HB + (r-1), so all three sy shifts are free-dim
    offsets (the halo rows hold the neighboring h8 block's edge data).
  - Products on DVE (bf16 2x) with ~7 shifts/row offloaded to the idle
    Pool engine; channel reduction on the PE via a fixed block-diagonal
    selection matmul packing (k,h8) into 128/88-row PSUM tiles. PE does
    A-tile shifts then B-tile shifts per row so A drains overlap B
    matmuls; within each phase column-groups round-robin so weight loads
    overlap streaming.
  - corr stays resident in SBUF (no DRAM spill); ACT drains PSUM->SBUF
    capturing squeeze partials via accum_out.
  - Squeeze allreduce split: rows 0..HB-2 reduced early (latency hidden
    under the last row), last row folded into a second tiny allreduce.
  - Gated writeout straight from SBUF: A rows on ACT (per-partition
    scale), B rows on DVE (4x tensor_scalar), per-row output DMAs.
"""

import sys
import types

import numpy as np
import ml_dtypes


def _install_ntff_hook_shim():
    """agent image's antenv lacks axon_hooks; needed only for trace=True."""
    if "antenv.axon_hooks" in sys.modules:
        return
    try:
        import antenv
        from trn_agent_boot.trn_boot import _ntff_profile_via_ctypes

        hook = _ntff_profile_via_ctypes("/opt/axon/libaxon_pjrt.so")
        mod = types.ModuleType("antenv.axon_hooks")
        ref = {"h": hook}
        mod.get_axon_ntff_profile_hook = lambda: ref["h"]
        mod.set_axon_ntff_profile_hook = lambda h: ref.__setitem__("h", h)
        sys.modules["antenv.axon_hooks"] = mod
        antenv.axon_hooks = mod
    except Exception:
        pass


_install_ntff_hook_shim()

import concourse.bacc as bacc  # noqa: E402
import concourse.tile as tile  # noqa: E402
import concourse.mybir as mybir  # noqa: E402
from concourse.bass_utils import run_bass_kernel_spmd  # noqa: E402

BF16 = mybir.dt.bfloat16
FP32 = mybir.dt.float32
AF = mybir.ActivationFunctionType
ALU = mybir.AluOpType

N_CORES = 8
C = 16
H8 = 8          # partition sub-dim over h
K = 27
MID = 6

# shifts whose products run on the Pool engine (DVE handles the rest).
# Empty: Pool's software tensor_tensor is ~3us/row-product AND its SBUF
# reads contend with DVE, knocking DVE products out of 2x mode.
POOL_SHIFTS = frozenset()
POOL_STT = False  # walrus rejects scalar_tensor_tensor on Pool


class Cfg:
    def __init__(self, W=80, H=96, D=112):
        assert H % H8 == 0
        self.W, self.H, self.D = W, H, D
        self.Wc = W // N_CORES          # w columns per core
        self.HB = H // H8               # hblk extent (free dim)
        self.De = D + 2                 # odd-copy d extent
        self.FD = self.Wc * D           # flat (w, d) free size per row
        self.slices = [(o, min(o + 512, self.FD))
                       for o in range(0, self.FD, 512)]
        self.n_fs = len(self.slices)
        assert self.HB % 2 == 0 and self.HB >= 2
        self.groups = [(j, 1) for j in range(self.HB)]


# shift order matches reference: k = dx*9 + dy*3 + dz, s* = d*-1
SHIFTS = [(dx - 1, dy - 1, dz - 1)
          for dx in range(3) for dy in range(3) for dz in range(3)]

# PE consumption order: zip the tile-A chain (PSUM banks psA*) with the
# tile-B chain (banks psB*) so consecutive matmuls alternate banks and
# mostly alternate PE column groups, while each bank keeps a single open
# accumulation group at a time. B starts at group 1 to de-align positions.
_A_CHAIN = [4 * g + v for g in range(4) for v in range(4)]
_B_CHAIN = [16 + 4 * g + v for g in (1, 2, 0) for v in range(4 if g < 2 else 3)]
PE_ORDER = []
for _i in range(16):
    PE_ORDER.append(_A_CHAIN[_i])
    if _i < 11:
        PE_ORDER.append(_B_CHAIN[_i])


def _gv_of(k):
    """(is_A, psum column group, selection slice) for shift k."""
    kk = k if k < 16 else k - 16
    return k < 16, kk // 4, kk % 4


def _row_of(k, h8):
    """corr partition row for (k, h8). Tile A: k 0..15, tile B: 16..26."""
    kk = k if k < 16 else k - 16
    base = 0 if k < 16 else 128
    return base + 32 * (kk // 4) + 8 * (kk % 4) + h8


def build_nc(cfg: Cfg):
    nc = bacc.Bacc("TRN2", target_bir_lowering=False, debug=False,
                   num_devices=N_CORES)
    HB, Wc, D, De, FD = cfg.HB, cfg.Wc, cfg.D, cfg.De, cfg.FD

    x1_d = nc.dram_tensor("x1", [128, HB, Wc, D], BF16, kind="ExternalInput")
    x2e_d = nc.dram_tensor("x2e", [128, HB + 2, Wc + 2, D], BF16,
                           kind="ExternalInput")
    x2o_d = nc.dram_tensor("x2o", [128, HB + 2, Wc + 2, De], BF16,
                           kind="ExternalInput")
    sel_d = nc.dram_tensor("selmats", [128, 128], BF16, kind="ExternalInput")
    w0a_d = nc.dram_tensor("w0a", [128, MID], FP32, kind="ExternalInput")
    w0b_d = nc.dram_tensor("w0b", [88, MID], FP32, kind="ExternalInput")
    w1a_d = nc.dram_tensor("w1ra", [MID, 128], FP32, kind="ExternalInput")
    w1b_d = nc.dram_tensor("w1rb", [MID, 88], FP32, kind="ExternalInput")
    b0_d = nc.dram_tensor("b0c", [MID, 1], FP32, kind="ExternalInput")
    b1a_d = nc.dram_tensor("b1ra", [128, 1], FP32, kind="ExternalInput")
    b1b_d = nc.dram_tensor("b1rb", [88, 1], FP32, kind="ExternalInput")
    out_d = nc.dram_tensor("out", [216, HB, FD], BF16, kind="ExternalOutput")

    n_drain = HB * cfg.n_fs
    wh = (Wc + 2) // 2 or 1         # x2 w-half for split loads
    xh = max(Wc // 2, 1)            # x1 w-half

    with tile.TileContext(nc) as tc:
        with (
            tc.tile_pool(name="const", bufs=1) as cpool,
            tc.tile_pool(name="ps", bufs=1, space="PSUM") as ps,
            tc.tile_pool(name="dram", bufs=1, space="DRAM") as dram,
            tc.tile_pool(name="pp", bufs=5) as ppool,
            tc.tile_pool(name="qq", bufs=2) as qpool,
            tc.tile_pool(name="stage", bufs=2) as spool,
        ):
            # resident tiles
            x1t = cpool.tile([128, HB, Wc, D], BF16)
            x2e_t = cpool.tile([128, HB + 2, Wc + 2, D], BF16)
            x2o_t = cpool.tile([128, HB + 2, Wc + 2, De], BF16)
            corrA = cpool.tile([128, HB, FD], BF16)
            corrB = cpool.tile([88, HB, FD], BF16)
            selt = cpool.tile([128, 128], BF16)
            w0at = cpool.tile([128, MID], FP32)
            w0bt = cpool.tile([88, MID], FP32)
            w1at = cpool.tile([MID, 128], FP32)
            w1bt = cpool.tile([MID, 88], FP32)
            b0t = cpool.tile([MID, 1], FP32)
            b1at = cpool.tile([128, 1], FP32)
            b1bt = cpool.tile([88, 1], FP32)
            accA = cpool.tile([128, n_drain], FP32)
            accB = cpool.tile([88, n_drain], FP32)

            nc.sync.dma_start(selt[:], sel_d[:])
            nc.sync.dma_start(w0at[:], w0a_d[:])
            nc.sync.dma_start(w0bt[:], w0b_d[:])
            nc.sync.dma_start(w1at[:], w1a_d[:])
            nc.sync.dma_start(w1bt[:], w1b_d[:])
            nc.sync.dma_start(b0t[:], b0_d[:])
            nc.sync.dma_start(b1at[:], b1a_d[:])
            nc.sync.dma_start(b1bt[:], b1b_d[:])

            def load_x1_row(r):
                nc.sync.dma_start(x1t[:, r, 0:xh, :], x1_d[:, r, 0:xh, :])
                if xh < Wc:
                    nc.sync.dma_start(x1t[:, r, xh:Wc, :],
                                      x1_d[:, r, xh:Wc, :])

            def load_x2_row(r, parts=2):
                cuts = [round(i * (Wc + 2) / parts) for i in range(parts + 1)]
                for t, d in ((x2e_t, x2e_d), (x2o_t, x2o_d)):
                    for a, b in zip(cuts, cuts[1:]):
                        if a < b:
                            nc.sync.dma_start(t[:, r, a:b, :], d[:, r, a:b, :])

            # priority-ordered input loads: first rows first, finest first
            load_x1_row(0)
            for r in range(min(2, HB + 2)):
                load_x2_row(r, parts=4)
            if HB > 1:
                load_x1_row(1)
            for r in range(2, min(4, HB + 2)):
                load_x2_row(r, parts=2)

            # Warm-up collective: absorbs cross-core launch skew and CC
            # firmware setup so the real allreduces only pay marginal latency.
            warm_in = dram.tile([MID, 1], FP32)
            warm_out = dram.tile([MID, 1], FP32)
            nc.sync.dma_start(warm_in[:], b0_d[:])
            nc.gpsimd.collective_compute(
                "AllReduce", ALU.add,
                replica_groups=[list(range(N_CORES))],
                ins=[warm_in[:].opt()],
                outs=[warm_out[:].opt()],
            )

            # remaining loads, interleaved in order of first use
            nx1 = 2
            for r in range(4, HB + 2, 2):
                while nx1 < min(r - 1, HB):
                    load_x1_row(nx1)
                    nx1 += 1
                load_x2_row(r)
                if r + 1 < HB + 2:
                    load_x2_row(r + 1)
            while nx1 < HB:
                load_x1_row(nx1)
                nx1 += 1

            cc_in = dram.tile([216, 1], FP32)
            cc_out = dram.tile([216, 1], FP32)

            a_tot = {g: 4 for g in range(4)}
            b_tot = {0: 4, 1: 4, 2: 3}

            def emit_products(j):
                """products for all 27 shifts of row j."""
                prods = {}
                pool_ks = [k for k in PE_ORDER if k in POOL_SHIFTS]
                dve_ks = [k for k in PE_ORDER if k not in POOL_SHIFTS]
                for k in pool_ks + dve_ks:
                    sx, sy, sz = SHIFTS[k]
                    if sz == 0:
                        src = x2e_t[:, 1 + j + sy, 1 + sx:1 + sx + Wc, 0:D]
                    else:
                        doff = sz + 1
                        src = x2o_t[:, 1 + j + sy, 1 + sx:1 + sx + Wc,
                                    doff:doff + D]
                    x1s = x1t[:, j, :, :]
                    if k in POOL_SHIFTS:
                        pt = qpool.tile([128, FD], BF16, tag="Q", bufs=3)
                        dst = pt.rearrange("p (w d) -> p w d", d=D)
                        if POOL_STT:
                            nc.gpsimd.scalar_tensor_tensor(
                                dst, x1s, 1.0, src, ALU.mult, ALU.mult)
                        else:
                            nc.gpsimd.tensor_tensor(dst, x1s, src, ALU.mult)
                    else:
                        pt = ppool.tile([128, FD], BF16, tag="P", bufs=5)
                        dst = pt.rearrange("p (w d) -> p w d", d=D)
                        nc.vector.tensor_tensor(dst, x1s, src, ALU.mult)
                    prods[k] = pt
                return prods

            def emit_reduce_row(j, prods):
                """PE reduction + ACT drains for row j."""
                psA = [ps.tile([128, hi - lo], FP32, tag=f"psA{i}",
                               name=f"psA{i}", padded_shape=[128, 512])
                       for i, (lo, hi) in enumerate(cfg.slices)]
                psB = [ps.tile([128, hi - lo], FP32, tag=f"psB{i}",
                               name=f"psB{i}", padded_shape=[128, 512])
                       for i, (lo, hi) in enumerate(cfg.slices)]
                seen = {}
                for k in PE_ORDER:
                    is_a, g, v = _gv_of(k)
                    tot = a_tot[g] if is_a else b_tot[g]
                    cnt = seen.get((is_a, g), 0)
                    seen[(is_a, g)] = cnt + 1
                    pst = psA if is_a else psB
                    for i, (lo, hi) in enumerate(cfg.slices):
                        nc.tensor.matmul(
                            pst[i][32 * g:32 * g + 32, :],
                            selt[:, 32 * v:32 * v + 32],
                            prods[k][:, lo:hi],
                            start=(cnt == 0), stop=(cnt == tot - 1),
                            tile_position=(0, 32 * g),
                        )
                    if k == _B_CHAIN[-1]:
                        # B chain done while A's tail streams: drain B now
                        for i, (lo, hi) in enumerate(cfg.slices):
                            di = j * cfg.n_fs + i
                            nc.scalar.activation(
                                corrB[:, j, lo:hi], psB[i][0:88, :], AF.Copy,
                                accum_out=accB[:, di:di + 1])
                for i, (lo, hi) in enumerate(cfg.slices):
                    di = j * cfg.n_fs + i
                    nc.scalar.activation(
                        corrA[:, j, lo:hi], psA[i][:], AF.Copy,
                        accum_out=accA[:, di:di + 1])

            for gi, (j0, nr) in enumerate(cfg.groups):
                prods = emit_products(j0)
                emit_reduce_row(j0, prods)

            # ---- squeeze partials + single allreduce + gate MLP ----
            pA = cpool.tile([128, 1], FP32)
            pB = cpool.tile([88, 1], FP32)
            scrA = cpool.tile([128, n_drain], FP32)
            scrB = cpool.tile([88, n_drain], FP32)
            nc.scalar.activation(scrA[:], accA[:, 0:n_drain],
                                 AF.Copy, accum_out=pA[:])
            nc.scalar.activation(scrB[:], accB[:, 0:n_drain],
                                 AF.Copy, accum_out=pB[:])
            nc.sync.dma_start(cc_in[0:128, :], pA[:])
            nc.sync.dma_start(cc_in[128:216, :], pB[:])
            nc.gpsimd.collective_compute(
                "AllReduce", ALU.add,
                replica_groups=[list(range(N_CORES))],
                ins=[cc_in[:].opt()],
                outs=[cc_out[:].opt()],
            )
            pAg = cpool.tile([128, 1], FP32)
            pBg = cpool.tile([88, 1], FP32)
            nc.sync.dma_start(pAg[:], cc_out[0:128, :])
            nc.sync.dma_start(pBg[:], cc_out[128:216, :])

            hps = ps.tile([MID, 1], FP32, tag="psA0", padded_shape=[128, 512])
            nc.tensor.matmul(hps[:], w0at[:], pAg[:], start=True, stop=False)
            nc.tensor.matmul(hps[:], w0bt[:], pBg[:], start=False, stop=True)
            hvec = cpool.tile([MID, 1], FP32)
            nc.scalar.activation(hvec[:], hps[:], AF.Relu, bias=b0t[:],
                                 scale=1.0)
            gpsA = ps.tile([128, 1], FP32, tag="psA1", padded_shape=[128, 512])
            gpsB = ps.tile([88, 1], FP32, tag="psA2", padded_shape=[128, 512])
            nc.tensor.matmul(gpsA[:], w1at[:], hvec[:], start=True, stop=True)
            nc.tensor.matmul(gpsB[:], w1bt[:], hvec[:], start=True, stop=True)
            gA = cpool.tile([128, 1], FP32)
            gB = cpool.tile([88, 1], FP32)
            nc.scalar.activation(gA[:], gpsA[:], AF.Sigmoid, bias=b1at[:],
                                 scale=1.0)
            nc.scalar.activation(gB[:], gpsB[:], AF.Sigmoid, bias=b1bt[:],
                                 scale=1.0)

            # ---- gated writeout from SBUF (A on ACT, B on DVE 4x).
            # Half-row output DMAs, all issued from the idle Pool sequencer
            # (cheapest DGE dispatch) to keep ACT/SP free for gating. ----
            fh = (FD // 2 + 1) & ~1 if FD > 2 else FD
            for j in range(HB):
                stA = spool.tile([128, FD], BF16, tag="gsA", bufs=4)
                nc.scalar.mul(stA[:], corrA[:, j, :], gA[:])
                stB = spool.tile([88, FD], BF16, tag="gsB", bufs=6)
                nc.vector.tensor_scalar(stB[:], corrB[:, j, :], gB[:],
                                        None, ALU.mult)
                nc.gpsimd.dma_start(out_d[0:128, j, 0:fh], stA[:, 0:fh])
                nc.gpsimd.dma_start(out_d[0:128, j, fh:FD], stA[:, fh:FD])
                nc.gpsimd.dma_start(out_d[128:216, j, 0:fh], stB[:, 0:fh])
                nc.gpsimd.dma_start(out_d[128:216, j, fh:FD], stB[:, fh:FD])

    nc.compile()
    return nc


# ---------------- host-side prep / assembly ----------------

def make_gate_consts(w0, b0, w1, b1, cfg: Cfg):
    norm = 1.0 / (cfg.W * cfg.H * cfg.D)
    sel = np.zeros((128, 128), dtype=np.float32)
    for v in range(4):
        for c in range(C):
            for h8 in range(H8):
                sel[c * H8 + h8, 32 * v + 8 * v + h8] = 1.0 / 16
    w0 = np.asarray(w0, dtype=np.float32)
    w1 = np.asarray(w1, dtype=np.float32)
    b1 = np.asarray(b1, dtype=np.float32)
    w0a = np.zeros((128, MID), dtype=np.float32)
    w0b = np.zeros((88, MID), dtype=np.float32)
    w1ra = np.zeros((MID, 128), dtype=np.float32)
    w1rb = np.zeros((MID, 88), dtype=np.float32)
    b1ra = np.zeros((128, 1), dtype=np.float32)
    b1rb = np.zeros((88, 1), dtype=np.float32)
    for k in range(K):
        for h8 in range(H8):
            r = _row_of(k, h8)
            if k < 16:
                w0a[r, :] = w0[:, k] * norm
                w1ra[:, r] = w1[k, :]
                b1ra[r, 0] = b1[k]
            else:
                w0b[r - 128, :] = w0[:, k] * norm
                w1rb[:, r - 128] = w1[k, :]
                b1rb[r - 128, 0] = b1[k]
    return {
        "selmats": sel.astype(ml_dtypes.bfloat16),
        "w0a": w0a, "w0b": w0b, "w1ra": w1ra, "w1rb": w1rb,
        "b0c": np.asarray(b0, dtype=np.float32).reshape(MID, 1),
        "b1ra": b1ra, "b1rb": b1rb,
    }


def _fold(a, HB):
    # [C, w, H, D'] -> [(c h8), hblk, w, d]
    Cc, ww, hh, dd = a.shape
    a = a.reshape(Cc, ww, H8, HB, dd)
    a = np.ascontiguousarray(a.transpose(0, 2, 3, 1, 4))
    return a.reshape(C * H8, HB, ww, dd)


def make_inputs_per_core(x_1, x_2, w0, b0, w1, b1, cfg: Cfg):
    """x_1/x_2: [1, C, W, H, D] float32 -> list of per-core input dicts."""
    W, H, D, De = cfg.W, cfg.H, cfg.D, cfg.De
    Wc, HB = cfg.Wc, cfg.HB
    x1 = np.asarray(x_1)[0].astype(ml_dtypes.bfloat16)      # [C, W, H, D]
    x2 = np.asarray(x_2)[0].astype(ml_dtypes.bfloat16)
    # padded x2: w +-1, h +-1, d in [-1, D+1)
    x2p = np.zeros((C, W + 2, H + 2, D + 2), dtype=ml_dtypes.bfloat16)
    x2p[:, 1:W + 1, 1:H + 1, 1:D + 1] = x2
    # hblk-extended h indices: row r of (h8) block = x2p h-index h8*HB + r,
    # covering h = h8*HB - 1 .. (h8+1)*HB (1-voxel halo on both sides)
    hidx = (np.arange(H8) * HB)[:, None] + np.arange(HB + 2)  # [H8, HB+2]

    consts = make_gate_consts(w0, b0, w1, b1, cfg)
    in_maps = []
    for ci in range(N_CORES):
        ws = ci * Wc
        m = dict(consts)
        m["x1"] = _fold(x1[:, ws:ws + Wc, :, :], HB)
        blk = x2p[:, ws:ws + Wc + 2, :, :]                  # [C, Wc+2, H+2, De]
        ee = blk[:, :, hidx, 1:1 + D]                       # [C, Wc+2, H8, HB+2, D]
        oo = blk[:, :, hidx, 0:De]
        m["x2e"] = np.ascontiguousarray(
            ee.transpose(0, 2, 3, 1, 4)).reshape(128, HB + 2, Wc + 2, D)
        m["x2o"] = np.ascontiguousarray(
            oo.transpose(0, 2, 3, 1, 4)).reshape(128, HB + 2, Wc + 2, De)
        in_maps.append(m)
    return in_maps


def assemble_output(results, cfg: Cfg):
    W, H, D = cfg.W, cfg.H, cfg.D
    Wc, HB = cfg.Wc, cfg.HB
    rows = np.empty((K, H8), dtype=np.int64)
    for k in range(K):
        for h8 in range(H8):
            rows[k, h8] = _row_of(k, h8)
    out = np.empty((W, H, D, K), dtype=np.float32)
    for ci, r in enumerate(results):
        o = np.asarray(r["out"]).reshape(216, HB, Wc, D)
        core = o[rows]                        # [K, H8, HB, Wc, D]
        core = core.transpose(3, 1, 2, 4, 0)  # [Wc, H8, HB, D, K]
        out[ci * Wc:(ci + 1) * Wc] = core.reshape(Wc, H, D, K)
    return out[None]


_CACHE = {}
TRACE = False           # test harness can set kernel.TRACE = True


def kernel(x_1, x_2, w0, b0, w1, b1):
    cfg = Cfg()
    if "nc" not in _CACHE:
        _CACHE["nc"] = build_nc(cfg)
    nc = _CACHE["nc"]
    in_maps = make_inputs_per_core(x_1, x_2, w0, b0, w1, b1, cfg)
    last_exc = None
    for _attempt in range(3):
        try:
            res = run_bass_kernel_spmd(nc, in_maps,
                                       core_ids=list(range(N_CORES)),
                                       trace=TRACE)
            break
        except Exception as e:  # transient NRT device errors: retry
            last_exc = e
    else:
        raise last_exc
    _CACHE["last_res"] = res
    return assemble_output(res.results, cfg)


# revision 21
# speedup vs baseline: 1.1331x; 1.0249x over previous
"""Trainium2 Bass kernel for shifted-window correlation (27 shifts) + SE gate.

Reference computation (shapes hardcoded; B=1, C=16, W=80, H=96, D=112):
  corr[w,h,d,k] = mean_c x1[c,w,h,d] * x2[c, w+sx, h+sy, d+sz]   (zero-padded)
  s = mean_{w,h,d} corr;  g = sigmoid(w1 @ relu(w0 @ s + b0) + b1)
  out = corr * g

Strategy (8 cores, W sharded 10/core):
  - SBUF partition dim = (c:16, h8:8) where h8 = h // (H/8).
  - x2 loaded ONCE per parity (even/odd d for bf16 4B alignment) as a
    [128, HB+2, Wc+2, D(+2)] tile whose hblk axis carries a 1-row halo:
    row r holds h = h8*HB + (r-1), so all three sy shifts are free-dim
    offsets (the halo rows hold the neighboring h8 block's edge data).
  - Products on DVE (bf16 2x) with ~7 shifts/row offloaded to the idle
    Pool engine; channel reduction on the PE via a fixed block-diagonal
    selection matmul packing (k,h8) into 128/88-row PSUM tiles. PE does
    A-tile shifts then B-tile shifts per row so A drains overlap B
    matmuls; within each phase column-groups round-robin so weight loads
    overlap streaming.
  - corr stays resident in SBUF (no DRAM spill); ACT drains PSUM->SBUF
    capturing squeeze partials via accum_out.
  - Squeeze allreduce split: rows 0..HB-2 reduced early (latency hidden
    under the last row), last row folded into a second tiny allreduce.
  - Gated writeout straight from SBUF: A rows on ACT (per-partition
    scale), B rows on DVE (4x tensor_scalar), per-row output DMAs.
"""

import sys
import types

import numpy as np
import ml_dtypes


def _install_ntff_hook_shim():
    """agent image's antenv lacks axon_hooks; needed only for trace=True."""
    if "antenv.axon_hooks" in sys.modules:
        return
    try:
        import antenv
        from trn_agent_boot.trn_boot import _ntff_profile_via_ctypes

        hook = _ntff_profile_via_ctypes("/opt/axon/libaxon_pjrt.so")
        mod = types.ModuleType("antenv.axon_hooks")
        ref = {"h": hook}
        mod.get_axon_ntff_profile_hook = lambda: ref["h"]
        mod.set_axon_ntff_profile_hook = lambda h: ref.__setitem__("h", h)
        sys.modules["antenv.axon_hooks"] = mod
        antenv.axon_hooks = mod
    except Exception:
        pass


_install_ntff_hook_shim()

import concourse.bacc as bacc  # noqa: E402
import concourse.tile as tile  # noqa: E402
import concourse.mybir as mybir  # noqa: E402
from concourse.bass_utils import run_bass_kernel_spmd  # noqa: E402

BF16 = mybir.dt.bfloat16
FP32 = mybir.dt.float32
AF = mybir.ActivationFunctionType
ALU = mybir.AluOpType

N_CORES = 8
C = 16
H8 = 8          # partition sub-dim over h
K = 27
MID = 6

# shifts whose products run on the Pool engine (DVE handles the rest).
# Empty: Pool's software tensor_tensor is ~3us/row-product AND its SBUF
# reads contend with DVE, knocking DVE products out of 2x mode.
POOL_SHIFTS = frozenset()
POOL_STT = False  # walrus rejects scalar_tensor_tensor on Pool


class Cfg:
    def __init__(self, W=80, H=96, D=112):
        assert H % H8 == 0
        self.W, self.H, self.D = W, H, D
        self.Wc = W // N_CORES          # w columns per core
        self.HB = H // H8               # hblk extent (free dim)
        self.De = D + 2                 # odd-copy d extent
        self.FD = self.Wc * D           # flat (w, d) free size per row
        self.slices = [(o, min(o + 512, self.FD))
                       for o in range(0, self.FD, 512)]
        self.n_fs = len(self.slices)
        assert self.HB % 2 == 0 and self.HB >= 2
        self.groups = [(j, 1) for j in range(self.HB)]


# shift order matches reference: k = dx*9 + dy*3 + dz, s* = d*-1
SHIFTS = [(dx - 1, dy - 1, dz - 1)
          for dx in range(3) for dy in range(3) for dz in range(3)]

# PE consumption order: zip the tile-A chain (PSUM banks psA*) with the
# tile-B chain (banks psB*) so consecutive matmuls alternate banks and
# mostly alternate PE column groups, while each bank keeps a single open
# accumulation group at a time. B starts at group 1 to de-align positions.
_A_CHAIN = [4 * g + v for g in range(4) for v in range(4)]
_B_CHAIN = [16 + 4 * g + v for g in (1, 2, 0) for v in range(4 if g < 2 else 3)]
PE_ORDER = []
for _i in range(16):
    PE_ORDER.append(_A_CHAIN[_i])
    if _i < 11:
        PE_ORDER.append(_B_CHAIN[_i])


def _gv_of(k):
    """(is_A, psum column group, selection slice) for shift k."""
    kk = k if k < 16 else k - 16
    return k < 16, kk // 4, kk % 4


def _row_of(k, h8):
    """corr partition row for (k, h8). Tile A: k 0..15, tile B: 16..26."""
    kk = k if k < 16 else k - 16
    base = 0 if k < 16 else 128
    return base + 32 * (kk // 4) + 8 * (kk % 4) + h8


def build_nc(cfg: Cfg):
    nc = bacc.Bacc("TRN2", target_bir_lowering=False, debug=False,
                   num_devices=N_CORES)
    HB, Wc, D, De, FD = cfg.HB, cfg.Wc, cfg.D, cfg.De, cfg.FD

    x1_d = nc.dram_tensor("x1", [128, HB, Wc, D], BF16, kind="ExternalInput")
    x2e_d = nc.dram_tensor("x2e", [128, HB + 2, Wc + 2, D], BF16,
                           kind="ExternalInput")
    x2o_d = nc.dram_tensor("x2o", [128, HB + 2, Wc + 2, De], BF16,
                           kind="ExternalInput")
    sel_d = nc.dram_tensor("selmats", [128, 128], BF16, kind="ExternalInput")
    w0a_d = nc.dram_tensor("w0a", [128, MID], FP32, kind="ExternalInput")
    w0b_d = nc.dram_tensor("w0b", [88, MID], FP32, kind="ExternalInput")
    w1a_d = nc.dram_tensor("w1ra", [MID, 128], FP32, kind="ExternalInput")
    w1b_d = nc.dram_tensor("w1rb", [MID, 88], FP32, kind="ExternalInput")
    b0_d = nc.dram_tensor("b0c", [MID, 1], FP32, kind="ExternalInput")
    b1a_d = nc.dram_tensor("b1ra", [128, 1], FP32, kind="ExternalInput")
    b1b_d = nc.dram_tensor("b1rb", [88, 1], FP32, kind="ExternalInput")
    out_d = nc.dram_tensor("out", [216, HB, FD], BF16, kind="ExternalOutput")

    n_drain = HB * cfg.n_fs
    wh = (Wc + 2) // 2 or 1         # x2 w-half for split loads
    xh = max(Wc // 2, 1)            # x1 w-half

    with tile.TileContext(nc) as tc:
        with (
            tc.tile_pool(name="const", bufs=1) as cpool,
            tc.tile_pool(name="ps", bufs=1, space="PSUM") as ps,
            tc.tile_pool(name="dram", bufs=1, space="DRAM") as dram,
            tc.tile_pool(name="pp", bufs=5) as ppool,
            tc.tile_pool(name="qq", bufs=2) as qpool,
            tc.tile_pool(name="stage", bufs=2) as spool,
        ):
            # resident tiles
            x1t = cpool.tile([128, HB, Wc, D], BF16)
            x2e_t = cpool.tile([128, HB + 2, Wc + 2, D], BF16)
            x2o_t = cpool.tile([128, HB + 2, Wc + 2, De], BF16)
            corrA = cpool.tile([128, HB, FD], BF16)
            corrB = cpool.tile([88, HB, FD], BF16)
            selt = cpool.tile([128, 128], BF16)
            w0at = cpool.tile([128, MID], FP32)
            w0bt = cpool.tile([88, MID], FP32)
            w1at = cpool.tile([MID, 128], FP32)
            w1bt = cpool.tile([MID, 88], FP32)
            b0t = cpool.tile([MID, 1], FP32)
            b1at = cpool.tile([128, 1], FP32)
            b1bt = cpool.tile([88, 1], FP32)
            accA = cpool.tile([128, n_drain], FP32)
            accB = cpool.tile([88, n_drain], FP32)

            nc.sync.dma_start(selt[:], sel_d[:])
            nc.sync.dma_start(w0at[:], w0a_d[:])
            nc.sync.dma_start(w0bt[:], w0b_d[:])
            nc.sync.dma_start(w1at[:], w1a_d[:])
            nc.sync.dma_start(w1bt[:], w1b_d[:])
            nc.sync.dma_start(b0t[:], b0_d[:])
            nc.sync.dma_start(b1at[:], b1a_d[:])
            nc.sync.dma_start(b1bt[:], b1b_d[:])

            def load_x1_row(r, parts=2):
                cuts = [round(i * Wc / parts) for i in range(parts + 1)]
                for a, b in zip(cuts, cuts[1:]):
                    if a < b:
                        nc.sync.dma_start(x1t[:, r, a:b, :],
                                          x1_d[:, r, a:b, :])

            def load_x2_row(r, parts=2):
                cuts = [round(i * (Wc + 2) / parts) for i in range(parts + 1)]
                for t, d in ((x2o_t, x2o_d), (x2e_t, x2e_d)):
                    for a, b in zip(cuts, cuts[1:]):
                        if a < b:
                            nc.sync.dma_start(t[:, r, a:b, :], d[:, r, a:b, :])

            # priority-ordered input loads: first rows first, finest first
            load_x1_row(0)
            for r in range(min(2, HB + 2)):
                load_x2_row(r, parts=4)
            if HB > 1:
                load_x1_row(1)
            for r in range(2, min(4, HB + 2)):
                load_x2_row(r, parts=2)

            # Warm-up collective: absorbs cross-core launch skew and CC
            # firmware setup so the real allreduces only pay marginal latency.
            warm_in = dram.tile([MID, 1], FP32)
            warm_out = dram.tile([MID, 1], FP32)
            nc.sync.dma_start(warm_in[:], b0_d[:])
            nc.gpsimd.collective_compute(
                "AllReduce", ALU.add,
                replica_groups=[list(range(N_CORES))],
                ins=[warm_in[:].opt()],
                outs=[warm_out[:].opt()],
            )

            # remaining loads, interleaved in order of first use
            nx1 = 2
            for r in range(4, HB + 2, 2):
                while nx1 < min(r - 1, HB):
                    load_x1_row(nx1)
                    nx1 += 1
                load_x2_row(r)
                if r + 1 < HB + 2:
                    load_x2_row(r + 1)
            while nx1 < HB:
                load_x1_row(nx1)
                nx1 += 1

            cc_in = dram.tile([216, 2], FP32)
            cc_out = dram.tile([216, 2], FP32)

            a_tot = {g: 4 for g in range(4)}
            b_tot = {0: 4, 1: 4, 2: 3}

            def emit_products(j, mid_hook=None):
                """products for all 27 shifts of row j."""
                prods = {}
                pool_ks = [k for k in PE_ORDER if k in POOL_SHIFTS]
                dve_ks = [k for k in PE_ORDER if k not in POOL_SHIFTS]
                for ki, k in enumerate(pool_ks + dve_ks):
                    if ki == 10 and mid_hook is not None:
                        mid_hook()
                    sx, sy, sz = SHIFTS[k]
                    if sz == 0:
                        src = x2e_t[:, 1 + j + sy, 1 + sx:1 + sx + Wc, 0:D]
                    else:
                        doff = sz + 1
                        src = x2o_t[:, 1 + j + sy, 1 + sx:1 + sx + Wc,
                                    doff:doff + D]
                    x1s = x1t[:, j, :, :]
                    if k in POOL_SHIFTS:
                        pt = qpool.tile([128, FD], BF16, tag="Q", bufs=3)
                        dst = pt.rearrange("p (w d) -> p w d", d=D)
                        if POOL_STT:
                            nc.gpsimd.scalar_tensor_tensor(
                                dst, x1s, 1.0, src, ALU.mult, ALU.mult)
                        else:
                            nc.gpsimd.tensor_tensor(dst, x1s, src, ALU.mult)
                    else:
                        pt = ppool.tile([128, FD], BF16, tag="P", bufs=5)
                        dst = pt.rearrange("p (w d) -> p w d", d=D)
                        nc.vector.tensor_tensor(dst, x1s, src, ALU.mult)
                    prods[k] = pt
                return prods

            def emit_reduce_row(j, prods):
                """PE reduction + ACT drains for row j."""
                psA = [ps.tile([128, hi - lo], FP32, tag=f"psA{i}",
                               name=f"psA{i}", padded_shape=[128, 512])
                       for i, (lo, hi) in enumerate(cfg.slices)]
                psB = [ps.tile([128, hi - lo], FP32, tag=f"psB{i}",
                               name=f"psB{i}", padded_shape=[128, 512])
                       for i, (lo, hi) in enumerate(cfg.slices)]
                seen = {}
                for k in PE_ORDER:
                    is_a, g, v = _gv_of(k)
                    tot = a_tot[g] if is_a else b_tot[g]
                    cnt = seen.get((is_a, g), 0)
                    seen[(is_a, g)] = cnt + 1
                    pst = psA if is_a else psB
                    for i, (lo, hi) in enumerate(cfg.slices):
                        nc.tensor.matmul(
                            pst[i][32 * g:32 * g + 32, :],
                            selt[:, 32 * v:32 * v + 32],
                            prods[k][:, lo:hi],
                            start=(cnt == 0), stop=(cnt == tot - 1),
                            tile_position=(0, 32 * g),
                        )
                    if k == _B_CHAIN[-1]:
                        # B chain done while A's tail streams: drain B now
                        for i, (lo, hi) in enumerate(cfg.slices):
                            di = j * cfg.n_fs + i
                            nc.scalar.activation(
                                corrB[:, j, lo:hi], psB[i][0:88, :], AF.Copy,
                                accum_out=accB[:, di:di + 1])
                for i, (lo, hi) in enumerate(cfg.slices):
                    di = j * cfg.n_fs + i
                    nc.scalar.activation(
                        corrA[:, j, lo:hi], psA[i][:], AF.Copy,
                        accum_out=accA[:, di:di + 1])

            early_cols = (HB - 1) * cfg.n_fs
            pAe = cpool.tile([128, 1], FP32)
            pBe = cpool.tile([88, 1], FP32)

            def early_squeeze():
                # rows 0..HB-2 partial sums, reduced on DVE between last-row
                # products and shipped early so the post-drain cc path only
                # handles the last row's 3 columns.
                nc.vector.tensor_reduce(pAe[:], accA[:, 0:early_cols],
                                        mybir.AxisListType.X, ALU.add)
                nc.vector.tensor_reduce(pBe[:], accB[:, 0:early_cols],
                                        mybir.AxisListType.X, ALU.add)
                nc.gpsimd.dma_start(cc_in[0:128, 0:1], pAe[:])
                nc.gpsimd.dma_start(cc_in[128:216, 0:1], pBe[:])

            for gi, (j0, nr) in enumerate(cfg.groups):
                hook = early_squeeze if gi == len(cfg.groups) - 1 else None
                prods = emit_products(j0, mid_hook=hook)
                emit_reduce_row(j0, prods)

            # ---- last-row partials + single allreduce + gate MLP ----
            pA = cpool.tile([128, 1], FP32)
            pB = cpool.tile([88, 1], FP32)
            scrA = cpool.tile([128, n_drain - early_cols], FP32)
            scrB = cpool.tile([88, n_drain - early_cols], FP32)
            nc.scalar.activation(scrA[:], accA[:, early_cols:n_drain],
                                 AF.Copy, accum_out=pA[:])
            nc.scalar.activation(scrB[:], accB[:, early_cols:n_drain],
                                 AF.Copy, accum_out=pB[:])
            nc.gpsimd.dma_start(cc_in[0:128, 1:2], pA[:])
            nc.gpsimd.dma_start(cc_in[128:216, 1:2], pB[:])
            nc.gpsimd.collective_compute(
                "AllReduce", ALU.add,
                replica_groups=[list(range(N_CORES))],
                ins=[cc_in[:].opt()],
                outs=[cc_out[:].opt()],
            )
            pAg = cpool.tile([128, 2], FP32)
            pBg = cpool.tile([88, 2], FP32)
            nc.gpsimd.dma_start(pAg[:], cc_out[0:128, :])
            nc.gpsimd.dma_start(pBg[:], cc_out[128:216, :])

            hps = ps.tile([MID, 1], FP32, tag="psA0", padded_shape=[128, 512])
            nc.tensor.matmul(hps[:], w0at[:], pAg[:, 0:1], start=True, stop=False)
            nc.tensor.matmul(hps[:], w0bt[:], pBg[:, 0:1], start=False, stop=False)
            nc.tensor.matmul(hps[:], w0at[:], pAg[:, 1:2], start=False, stop=False)
            nc.tensor.matmul(hps[:], w0bt[:], pBg[:, 1:2], start=False, stop=True)
            hvec = cpool.tile([MID, 1], FP32)
            nc.scalar.activation(hvec[:], hps[:], AF.Relu, bias=b0t[:],
                                 scale=1.0)
            gpsA = ps.tile([128, 1], FP32, tag="psA1", padded_shape=[128, 512])
            gpsB = ps.tile([88, 1], FP32, tag="psA2", padded_shape=[128, 512])
            nc.tensor.matmul(gpsA[:], w1at[:], hvec[:], start=True, stop=True)
            nc.tensor.matmul(gpsB[:], w1bt[:], hvec[:], start=True, stop=True)
            gA = cpool.tile([128, 1], FP32)
            gB = cpool.tile([88, 1], FP32)
            nc.scalar.activation(gA[:], gpsA[:], AF.Sigmoid, bias=b1at[:],
                                 scale=1.0)
            nc.scalar.activation(gB[:], gpsB[:], AF.Sigmoid, bias=b1bt[:],
                                 scale=1.0)

            # ---- gated writeout from SBUF (A on ACT, B on DVE 4x).
            # Half-row output DMAs, all issued from the idle Pool sequencer
            # (cheapest DGE dispatch) to keep ACT/SP free for gating. ----
            fh = (FD // 2 + 1) & ~1 if FD > 2 else FD
            act_rows = (HB * 2) // 3
            for j in range(HB):
                stA = spool.tile([128, FD], BF16, tag="gsA", bufs=4)
                if j < act_rows:
                    nc.scalar.mul(stA[:], corrA[:, j, :], gA[:])
                else:
                    nc.vector.tensor_scalar(stA[:], corrA[:, j, :], gA[:],
                                            None, ALU.mult)
                stB = spool.tile([88, FD], BF16, tag="gsB", bufs=6)
                nc.vector.tensor_scalar(stB[:], corrB[:, j, :], gB[:],
                                        None, ALU.mult)
                nc.gpsimd.dma_start(out_d[0:128, j, 0:fh], stA[:, 0:fh])
                nc.gpsimd.dma_start(out_d[0:128, j, fh:FD], stA[:, fh:FD])
                nc.gpsimd.dma_start(out_d[128:216, j, 0:fh], stB[:, 0:fh])
                nc.gpsimd.dma_start(out_d[128:216, j, fh:FD], stB[:, fh:FD])

    nc.compile()
    return nc


# ---------------- host-side prep / assembly ----------------

def make_gate_consts(w0, b0, w1, b1, cfg: Cfg):
    norm = 1.0 / (cfg.W * cfg.H * cfg.D)
    sel = np.zeros((128, 128), dtype=np.float32)
    for v in range(4):
        for c in range(C):
            for h8 in range(H8):
                sel[c * H8 + h8, 32 * v + 8 * v + h8] = 1.0 / 16
    w0 = np.asarray(w0, dtype=np.float32)
    w1 = np.asarray(w1, dtype=np.float32)
    b1 = np.asarray(b1, dtype=np.float32)
    w0a = np.zeros((128, MID), dtype=np.float32)
    w0b = np.zeros((88, MID), dtype=np.float32)
    w1ra = np.zeros((MID, 128), dtype=np.float32)
    w1rb = np.zeros((MID, 88), dtype=np.float32)
    b1ra = np.zeros((128, 1), dtype=np.float32)
    b1rb = np.zeros((88, 1), dtype=np.float32)
    for k in range(K):
        for h8 in range(H8):
            r = _row_of(k, h8)
            if k < 16:
                w0a[r, :] = w0[:, k] * norm
                w1ra[:, r] = w1[k, :]
                b1ra[r, 0] = b1[k]
            else:
                w0b[r - 128, :] = w0[:, k] * norm
                w1rb[:, r - 128] = w1[k, :]
                b1rb[r - 128, 0] = b1[k]
    return {
        "selmats": sel.astype(ml_dtypes.bfloat16),
        "w0a": w0a, "w0b": w0b, "w1ra": w1ra, "w1rb": w1rb,
        "b0c": np.asarray(b0, dtype=np.float32).reshape(MID, 1),
        "b1ra": b1ra, "b1rb": b1rb,
    }


def _fold(a, HB):
    # [C, w, H, D'] -> [(c h8), hblk, w, d]
    Cc, ww, hh, dd = a.shape
    a = a.reshape(Cc, ww, H8, HB, dd)
    a = np.ascontiguousarray(a.transpose(0, 2, 3, 1, 4))
    return a.reshape(C * H8, HB, ww, dd)


def make_inputs_per_core(x_1, x_2, w0, b0, w1, b1, cfg: Cfg):
    """x_1/x_2: [1, C, W, H, D] float32 -> list of per-core input dicts."""
    W, H, D, De = cfg.W, cfg.H, cfg.D, cfg.De
    Wc, HB = cfg.Wc, cfg.HB
    x1 = np.asarray(x_1)[0].astype(ml_dtypes.bfloat16)      # [C, W, H, D]
    x2 = np.asarray(x_2)[0].astype(ml_dtypes.bfloat16)
    # padded x2: w +-1, h +-1, d in [-1, D+1)
    x2p = np.zeros((C, W + 2, H + 2, D + 2), dtype=ml_dtypes.bfloat16)
    x2p[:, 1:W + 1, 1:H + 1, 1:D + 1] = x2
    # hblk-extended h indices: row r of (h8) block = x2p h-index h8*HB + r,
    # covering h = h8*HB - 1 .. (h8+1)*HB (1-voxel halo on both sides)
    hidx = (np.arange(H8) * HB)[:, None] + np.arange(HB + 2)  # [H8, HB+2]

    consts = make_gate_consts(w0, b0, w1, b1, cfg)
    in_maps = []
    for ci in range(N_CORES):
        ws = ci * Wc
        m = dict(consts)
        m["x1"] = _fold(x1[:, ws:ws + Wc, :, :], HB)
        blk = x2p[:, ws:ws + Wc + 2, :, :]                  # [C, Wc+2, H+2, De]
        ee = blk[:, :, hidx, 1:1 + D]                       # [C, Wc+2, H8, HB+2, D]
        oo = blk[:, :, hidx, 0:De]
        m["x2e"] = np.ascontiguousarray(
            ee.transpose(0, 2, 3, 1, 4)).reshape(128, HB + 2, Wc + 2, D)
        m["x2o"] = np.ascontiguousarray(
            oo.transpose(0, 2, 3, 1, 4)).reshape(128, HB + 2, Wc + 2, De)
        in_maps.append(m)
    return in_maps


def assemble_output(results, cfg: Cfg):
    W, H, D = cfg.W, cfg.H, cfg.D
    Wc, HB = cfg.Wc, cfg.HB
    rows = np.empty((K, H8), dtype=np.int64)
    for k in range(K):
        for h8 in range(H8):
            rows[k, h8] = _row_of(k, h8)
    out = np.empty((W, H, D, K), dtype=np.float32)
    for ci, r in enumerate(results):
        o = np.asarray(r["out"]).reshape(216, HB, Wc, D)
        core = o[rows]                        # [K, H8, HB, Wc, D]
        core = core.transpose(3, 1, 2, 4, 0)  # [Wc, H8, HB, D, K]
        out[ci * Wc:(ci + 1) * Wc] = core.reshape(Wc, H, D, K)
    return out[None]


_CACHE = {}
TRACE = False           # test harness can set kernel.TRACE = True


def kernel(x_1, x_2, w0, b0, w1, b1):
    cfg = Cfg()
    if "nc" not in _CACHE:
        _CACHE["nc"] = build_nc(cfg)
    nc = _CACHE["nc"]
    in_maps = make_inputs_per_core(x_1, x_2, w0, b0, w1, b1, cfg)
    last_exc = None
    for _attempt in range(3):
        try:
            res = run_bass_kernel_spmd(nc, in_maps,
                                       core_ids=list(range(N_CORES)),
                                       trace=TRACE)
            break
        except Exception as e:  # transient NRT device errors: retry
            last_exc = e
    else:
        raise last_exc
    _CACHE["last_res"] = res
    return assemble_output(res.results, cfg)


# revision 22
# speedup vs baseline: 1.1355x; 1.0021x over previous
"""Trainium2 Bass kernel for shifted-window correlation (27 shifts) + SE gate.

Reference computation (shapes hardcoded; B=1, C=16, W=80, H=96, D=112):
  corr[w,h,d,k] = mean_c x1[c,w,h,d] * x2[c, w+sx, h+sy, d+sz]   (zero-padded)
  s = mean_{w,h,d} corr;  g = sigmoid(w1 @ relu(w0 @ s + b0) + b1)
  out = corr * g

Strategy (8 cores, W sharded 10/core):
  - SBUF partition dim = (c:16, h8:8) where h8 = h // (H/8).
  - x2 loaded ONCE per parity (even/odd d for bf16 4B alignment) as a
    [128, HB+2, Wc+2, D(+2)] tile whose hblk axis carries a 1-row halo:
    row r holds h = h8*HB + (r-1), so all three sy shifts are free-dim
    offsets (the halo rows hold the neighboring h8 block's edge data).
  - Products on DVE (bf16 2x) with ~7 shifts/row offloaded to the idle
    Pool engine; channel reduction on the PE via a fixed block-diagonal
    selection matmul packing (k,h8) into 128/88-row PSUM tiles. PE does
    A-tile shifts then B-tile shifts per row so A drains overlap B
    matmuls; within each phase column-groups round-robin so weight loads
    overlap streaming.
  - corr stays resident in SBUF (no DRAM spill); ACT drains PSUM->SBUF
    capturing squeeze partials via accum_out.
  - Squeeze allreduce split: rows 0..HB-2 reduced early (latency hidden
    under the last row), last row folded into a second tiny allreduce.
  - Gated writeout straight from SBUF: A rows on ACT (per-partition
    scale), B rows on DVE (4x tensor_scalar), per-row output DMAs.
"""

import sys
import types

import numpy as np
import ml_dtypes


def _install_ntff_hook_shim():
    """agent image's antenv lacks axon_hooks; needed only for trace=True."""
    if "antenv.axon_hooks" in sys.modules:
        return
    try:
        import antenv
        from trn_agent_boot.trn_boot import _ntff_profile_via_ctypes

        hook = _ntff_profile_via_ctypes("/opt/axon/libaxon_pjrt.so")
        mod = types.ModuleType("antenv.axon_hooks")
        ref = {"h": hook}
        mod.get_axon_ntff_profile_hook = lambda: ref["h"]
        mod.set_axon_ntff_profile_hook = lambda h: ref.__setitem__("h", h)
        sys.modules["antenv.axon_hooks"] = mod
        antenv.axon_hooks = mod
    except Exception:
        pass


_install_ntff_hook_shim()

import concourse.bacc as bacc  # noqa: E402
import concourse.tile as tile  # noqa: E402
import concourse.mybir as mybir  # noqa: E402
from concourse.bass_utils import run_bass_kernel_spmd  # noqa: E402

BF16 = mybir.dt.bfloat16
FP32 = mybir.dt.float32
AF = mybir.ActivationFunctionType
ALU = mybir.AluOpType

N_CORES = 8
C = 16
H8 = 8          # partition sub-dim over h
K = 27
MID = 6

# shifts whose products run on the Pool engine (DVE handles the rest).
# Empty: Pool's software tensor_tensor is ~3us/row-product AND its SBUF
# reads contend with DVE, knocking DVE products out of 2x mode.
POOL_SHIFTS = frozenset()
POOL_STT = False  # walrus rejects scalar_tensor_tensor on Pool


class Cfg:
    def __init__(self, W=80, H=96, D=112):
        assert H % H8 == 0
        self.W, self.H, self.D = W, H, D
        self.Wc = W // N_CORES          # w columns per core
        self.HB = H // H8               # hblk extent (free dim)
        self.De = D + 2                 # odd-copy d extent
        self.FD = self.Wc * D           # flat (w, d) free size per row
        self.slices = [(o, min(o + 512, self.FD))
                       for o in range(0, self.FD, 512)]
        self.n_fs = len(self.slices)
        assert self.HB % 2 == 0 and self.HB >= 2
        self.groups = [(j, 1) for j in range(self.HB)]


# shift order matches reference: k = dx*9 + dy*3 + dz, s* = d*-1
SHIFTS = [(dx - 1, dy - 1, dz - 1)
          for dx in range(3) for dy in range(3) for dz in range(3)]

# PE consumption order: zip the tile-A chain (PSUM banks psA*) with the
# tile-B chain (banks psB*) so consecutive matmuls alternate banks and
# mostly alternate PE column groups, while each bank keeps a single open
# accumulation group at a time. B starts at group 1 to de-align positions.
_A_CHAIN = [4 * g + v for g in range(4) for v in range(4)]
_B_CHAIN = [16 + 4 * g + v for g in (1, 2, 0) for v in range(4 if g < 2 else 3)]
PE_ORDER = []
for _i in range(16):
    PE_ORDER.append(_A_CHAIN[_i])
    if _i < 11:
        PE_ORDER.append(_B_CHAIN[_i])


def _gv_of(k):
    """(is_A, psum column group, selection slice) for shift k."""
    kk = k if k < 16 else k - 16
    return k < 16, kk // 4, kk % 4


def _row_of(k, h8):
    """corr partition row for (k, h8). Tile A: k 0..15, tile B: 16..26."""
    kk = k if k < 16 else k - 16
    base = 0 if k < 16 else 128
    return base + 32 * (kk // 4) + 8 * (kk % 4) + h8


def build_nc(cfg: Cfg):
    nc = bacc.Bacc("TRN2", target_bir_lowering=False, debug=False,
                   num_devices=N_CORES)
    HB, Wc, D, De, FD = cfg.HB, cfg.Wc, cfg.D, cfg.De, cfg.FD

    x1_d = nc.dram_tensor("x1", [HB, 128, Wc, D], BF16, kind="ExternalInput")
    x2e_d = nc.dram_tensor("x2e", [HB + 2, 128, Wc + 2, D], BF16,
                           kind="ExternalInput")
    x2o_d = nc.dram_tensor("x2o", [HB + 2, 128, Wc + 2, De], BF16,
                           kind="ExternalInput")
    sel_d = nc.dram_tensor("selmats", [128, 128], BF16, kind="ExternalInput")
    w0a_d = nc.dram_tensor("w0a", [128, MID], FP32, kind="ExternalInput")
    w0b_d = nc.dram_tensor("w0b", [88, MID], FP32, kind="ExternalInput")
    w1a_d = nc.dram_tensor("w1ra", [MID, 128], FP32, kind="ExternalInput")
    w1b_d = nc.dram_tensor("w1rb", [MID, 88], FP32, kind="ExternalInput")
    b0_d = nc.dram_tensor("b0c", [MID, 1], FP32, kind="ExternalInput")
    b1a_d = nc.dram_tensor("b1ra", [128, 1], FP32, kind="ExternalInput")
    b1b_d = nc.dram_tensor("b1rb", [88, 1], FP32, kind="ExternalInput")
    out_d = nc.dram_tensor("out", [HB, 216, FD], BF16, kind="ExternalOutput")

    n_drain = HB * cfg.n_fs
    wh = (Wc + 2) // 2 or 1         # x2 w-half for split loads
    xh = max(Wc // 2, 1)            # x1 w-half

    with tile.TileContext(nc) as tc:
        with (
            tc.tile_pool(name="const", bufs=1) as cpool,
            tc.tile_pool(name="ps", bufs=1, space="PSUM") as ps,
            tc.tile_pool(name="dram", bufs=1, space="DRAM") as dram,
            tc.tile_pool(name="pp", bufs=5) as ppool,
            tc.tile_pool(name="qq", bufs=2) as qpool,
            tc.tile_pool(name="stage", bufs=2) as spool,
        ):
            # resident tiles
            x1t = cpool.tile([128, HB, Wc, D], BF16)
            x2e_t = cpool.tile([128, HB + 2, Wc + 2, D], BF16)
            x2o_t = cpool.tile([128, HB + 2, Wc + 2, De], BF16)
            corrA = cpool.tile([128, HB, FD], BF16)
            corrB = cpool.tile([88, HB, FD], BF16)
            selt = cpool.tile([128, 128], BF16)
            w0at = cpool.tile([128, MID], FP32)
            w0bt = cpool.tile([88, MID], FP32)
            w1at = cpool.tile([MID, 128], FP32)
            w1bt = cpool.tile([MID, 88], FP32)
            b0t = cpool.tile([MID, 1], FP32)
            b1at = cpool.tile([128, 1], FP32)
            b1bt = cpool.tile([88, 1], FP32)
            accA = cpool.tile([128, n_drain], FP32)
            accB = cpool.tile([88, n_drain], FP32)

            nc.sync.dma_start(selt[:], sel_d[:])
            nc.sync.dma_start(w0at[:], w0a_d[:])
            nc.sync.dma_start(w0bt[:], w0b_d[:])
            nc.sync.dma_start(w1at[:], w1a_d[:])
            nc.sync.dma_start(w1bt[:], w1b_d[:])
            nc.sync.dma_start(b0t[:], b0_d[:])
            nc.sync.dma_start(b1at[:], b1a_d[:])
            nc.sync.dma_start(b1bt[:], b1b_d[:])

            def load_x1_row(r, parts=2):
                cuts = [round(i * Wc / parts) for i in range(parts + 1)]
                for a, b in zip(cuts, cuts[1:]):
                    if a < b:
                        nc.sync.dma_start(x1t[:, r, a:b, :],
                                          x1_d[r, :, a:b, :])

            def load_x2_row(r, parts=2):
                cuts = [round(i * (Wc + 2) / parts) for i in range(parts + 1)]
                for t, d in ((x2o_t, x2o_d), (x2e_t, x2e_d)):
                    for a, b in zip(cuts, cuts[1:]):
                        if a < b:
                            nc.sync.dma_start(t[:, r, a:b, :], d[r, :, a:b, :])

            # priority-ordered input loads: first rows first, finest first
            load_x1_row(0)
            for r in range(min(2, HB + 2)):
                load_x2_row(r, parts=4)
            if HB > 1:
                load_x1_row(1)
            for r in range(2, min(4, HB + 2)):
                load_x2_row(r, parts=2)

            # Warm-up collective: absorbs cross-core launch skew and CC
            # firmware setup so the real allreduces only pay marginal latency.
            warm_in = dram.tile([MID, 1], FP32)
            warm_out = dram.tile([MID, 1], FP32)
            nc.sync.dma_start(warm_in[:], b0_d[:])
            nc.gpsimd.collective_compute(
                "AllReduce", ALU.add,
                replica_groups=[list(range(N_CORES))],
                ins=[warm_in[:].opt()],
                outs=[warm_out[:].opt()],
            )

            # remaining loads, interleaved in order of first use
            nx1 = 2
            for r in range(4, HB + 2, 2):
                while nx1 < min(r - 1, HB):
                    load_x1_row(nx1)
                    nx1 += 1
                load_x2_row(r)
                if r + 1 < HB + 2:
                    load_x2_row(r + 1)
            while nx1 < HB:
                load_x1_row(nx1)
                nx1 += 1

            cc_in = dram.tile([216, 2], FP32)
            cc_out = dram.tile([216, 2], FP32)

            a_tot = {g: 4 for g in range(4)}
            b_tot = {0: 4, 1: 4, 2: 3}

            def emit_products(j, mid_hook=None):
                """products for all 27 shifts of row j."""
                prods = {}
                pool_ks = [k for k in PE_ORDER if k in POOL_SHIFTS]
                dve_ks = [k for k in PE_ORDER if k not in POOL_SHIFTS]
                for ki, k in enumerate(pool_ks + dve_ks):
                    if ki == 10 and mid_hook is not None:
                        mid_hook()
                    sx, sy, sz = SHIFTS[k]
                    if sz == 0:
                        src = x2e_t[:, 1 + j + sy, 1 + sx:1 + sx + Wc, 0:D]
                    else:
                        doff = sz + 1
                        src = x2o_t[:, 1 + j + sy, 1 + sx:1 + sx + Wc,
                                    doff:doff + D]
                    x1s = x1t[:, j, :, :]
                    if k in POOL_SHIFTS:
                        pt = qpool.tile([128, FD], BF16, tag="Q", bufs=3)
                        dst = pt.rearrange("p (w d) -> p w d", d=D)
                        if POOL_STT:
                            nc.gpsimd.scalar_tensor_tensor(
                                dst, x1s, 1.0, src, ALU.mult, ALU.mult)
                        else:
                            nc.gpsimd.tensor_tensor(dst, x1s, src, ALU.mult)
                    else:
                        pt = ppool.tile([128, FD], BF16, tag="P", bufs=5)
                        dst = pt.rearrange("p (w d) -> p w d", d=D)
                        nc.vector.tensor_tensor(dst, x1s, src, ALU.mult)
                    prods[k] = pt
                return prods

            def emit_reduce_row(j, prods):
                """PE reduction + ACT drains for row j."""
                psA = [ps.tile([128, hi - lo], FP32, tag=f"psA{i}",
                               name=f"psA{i}", padded_shape=[128, 512])
                       for i, (lo, hi) in enumerate(cfg.slices)]
                psB = [ps.tile([128, hi - lo], FP32, tag=f"psB{i}",
                               name=f"psB{i}", padded_shape=[128, 512])
                       for i, (lo, hi) in enumerate(cfg.slices)]
                seen = {}
                for k in PE_ORDER:
                    is_a, g, v = _gv_of(k)
                    tot = a_tot[g] if is_a else b_tot[g]
                    cnt = seen.get((is_a, g), 0)
                    seen[(is_a, g)] = cnt + 1
                    pst = psA if is_a else psB
                    for i, (lo, hi) in enumerate(cfg.slices):
                        nc.tensor.matmul(
                            pst[i][32 * g:32 * g + 32, :],
                            selt[:, 32 * v:32 * v + 32],
                            prods[k][:, lo:hi],
                            start=(cnt == 0), stop=(cnt == tot - 1),
                            tile_position=(0, 32 * g),
                        )
                    if k == _B_CHAIN[-1]:
                        # B chain done while A's tail streams: drain B now
                        for i, (lo, hi) in enumerate(cfg.slices):
                            di = j * cfg.n_fs + i
                            nc.scalar.activation(
                                corrB[:, j, lo:hi], psB[i][0:88, :], AF.Copy,
                                accum_out=accB[:, di:di + 1])
                for i, (lo, hi) in enumerate(cfg.slices):
                    di = j * cfg.n_fs + i
                    nc.scalar.activation(
                        corrA[:, j, lo:hi], psA[i][:], AF.Copy,
                        accum_out=accA[:, di:di + 1])

            early_cols = (HB - 1) * cfg.n_fs
            pAe = cpool.tile([128, 1], FP32)
            pBe = cpool.tile([88, 1], FP32)

            def early_squeeze():
                # rows 0..HB-2 partial sums, reduced on DVE between last-row
                # products and shipped early so the post-drain cc path only
                # handles the last row's 3 columns.
                nc.vector.tensor_reduce(pAe[:], accA[:, 0:early_cols],
                                        mybir.AxisListType.X, ALU.add)
                nc.vector.tensor_reduce(pBe[:], accB[:, 0:early_cols],
                                        mybir.AxisListType.X, ALU.add)
                nc.gpsimd.dma_start(cc_in[0:128, 0:1], pAe[:])
                nc.gpsimd.dma_start(cc_in[128:216, 0:1], pBe[:])

            for gi, (j0, nr) in enumerate(cfg.groups):
                hook = early_squeeze if gi == len(cfg.groups) - 1 else None
                prods = emit_products(j0, mid_hook=hook)
                emit_reduce_row(j0, prods)

            # ---- last-row partials + single allreduce + gate MLP ----
            pA = cpool.tile([128, 1], FP32)
            pB = cpool.tile([88, 1], FP32)
            scrA = cpool.tile([128, n_drain - early_cols], FP32)
            scrB = cpool.tile([88, n_drain - early_cols], FP32)
            nc.scalar.activation(scrA[:], accA[:, early_cols:n_drain],
                                 AF.Copy, accum_out=pA[:])
            nc.scalar.activation(scrB[:], accB[:, early_cols:n_drain],
                                 AF.Copy, accum_out=pB[:])
            nc.gpsimd.dma_start(cc_in[0:128, 1:2], pA[:])
            nc.gpsimd.dma_start(cc_in[128:216, 1:2], pB[:])
            nc.gpsimd.collective_compute(
                "AllReduce", ALU.add,
                replica_groups=[list(range(N_CORES))],
                ins=[cc_in[:].opt()],
                outs=[cc_out[:].opt()],
            )
            pAg = cpool.tile([128, 2], FP32)
            pBg = cpool.tile([88, 2], FP32)
            nc.gpsimd.dma_start(pAg[:], cc_out[0:128, :])
            nc.gpsimd.dma_start(pBg[:], cc_out[128:216, :])

            hps = ps.tile([MID, 1], FP32, tag="psA0", padded_shape=[128, 512])
            nc.tensor.matmul(hps[:], w0at[:], pAg[:, 0:1], start=True, stop=False)
            nc.tensor.matmul(hps[:], w0bt[:], pBg[:, 0:1], start=False, stop=False)
            nc.tensor.matmul(hps[:], w0at[:], pAg[:, 1:2], start=False, stop=False)
            nc.tensor.matmul(hps[:], w0bt[:], pBg[:, 1:2], start=False, stop=True)
            hvec = cpool.tile([MID, 1], FP32)
            nc.scalar.activation(hvec[:], hps[:], AF.Relu, bias=b0t[:],
                                 scale=1.0)
            gpsA = ps.tile([128, 1], FP32, tag="psA1", padded_shape=[128, 512])
            gpsB = ps.tile([88, 1], FP32, tag="psA2", padded_shape=[128, 512])
            nc.tensor.matmul(gpsA[:], w1at[:], hvec[:], start=True, stop=True)
            nc.tensor.matmul(gpsB[:], w1bt[:], hvec[:], start=True, stop=True)
            gA = cpool.tile([128, 1], FP32)
            gB = cpool.tile([88, 1], FP32)
            nc.scalar.activation(gA[:], gpsA[:], AF.Sigmoid, bias=b1at[:],
                                 scale=1.0)
            nc.scalar.activation(gB[:], gpsB[:], AF.Sigmoid, bias=b1bt[:],
                                 scale=1.0)

            # ---- gated writeout from SBUF (A on ACT, B on DVE 4x).
            # Half-row output DMAs, all issued from the idle Pool sequencer
            # (cheapest DGE dispatch) to keep ACT/SP free for gating. ----
            fh = (FD // 2 + 1) & ~1 if FD > 2 else FD
            act_rows = (HB * 2) // 3
            for j in range(HB):
                stA = spool.tile([128, FD], BF16, tag="gsA", bufs=4)
                if j < act_rows:
                    nc.scalar.mul(stA[:], corrA[:, j, :], gA[:])
                else:
                    nc.vector.tensor_scalar(stA[:], corrA[:, j, :], gA[:],
                                            None, ALU.mult)
                stB = spool.tile([88, FD], BF16, tag="gsB", bufs=6)
                nc.vector.tensor_scalar(stB[:], corrB[:, j, :], gB[:],
                                        None, ALU.mult)
                nc.sync.dma_start(out_d[j, 0:128, 0:fh], stA[:, 0:fh])
                nc.sync.dma_start(out_d[j, 0:128, fh:FD], stA[:, fh:FD])
                nc.gpsimd.dma_start(out_d[j, 128:216, 0:fh], stB[:, 0:fh])
                nc.gpsimd.dma_start(out_d[j, 128:216, fh:FD], stB[:, fh:FD])

    nc.compile()
    return nc


# ---------------- host-side prep / assembly ----------------

def make_gate_consts(w0, b0, w1, b1, cfg: Cfg):
    norm = 1.0 / (cfg.W * cfg.H * cfg.D)
    sel = np.zeros((128, 128), dtype=np.float32)
    for v in range(4):
        for c in range(C):
            for h8 in range(H8):
                sel[c * H8 + h8, 32 * v + 8 * v + h8] = 1.0 / 16
    w0 = np.asarray(w0, dtype=np.float32)
    w1 = np.asarray(w1, dtype=np.float32)
    b1 = np.asarray(b1, dtype=np.float32)
    w0a = np.zeros((128, MID), dtype=np.float32)
    w0b = np.zeros((88, MID), dtype=np.float32)
    w1ra = np.zeros((MID, 128), dtype=np.float32)
    w1rb = np.zeros((MID, 88), dtype=np.float32)
    b1ra = np.zeros((128, 1), dtype=np.float32)
    b1rb = np.zeros((88, 1), dtype=np.float32)
    for k in range(K):
        for h8 in range(H8):
            r = _row_of(k, h8)
            if k < 16:
                w0a[r, :] = w0[:, k] * norm
                w1ra[:, r] = w1[k, :]
                b1ra[r, 0] = b1[k]
            else:
                w0b[r - 128, :] = w0[:, k] * norm
                w1rb[:, r - 128] = w1[k, :]
                b1rb[r - 128, 0] = b1[k]
    return {
        "selmats": sel.astype(ml_dtypes.bfloat16),
        "w0a": w0a, "w0b": w0b, "w1ra": w1ra, "w1rb": w1rb,
        "b0c": np.asarray(b0, dtype=np.float32).reshape(MID, 1),
        "b1ra": b1ra, "b1rb": b1rb,
    }


def _fold(a, HB):
    # [C, w, H, D'] -> [(c h8), hblk, w, d]
    Cc, ww, hh, dd = a.shape
    a = a.reshape(Cc, ww, H8, HB, dd)
    a = np.ascontiguousarray(a.transpose(0, 2, 3, 1, 4))
    return a.reshape(C * H8, HB, ww, dd)


def make_inputs_per_core(x_1, x_2, w0, b0, w1, b1, cfg: Cfg):
    """x_1/x_2: [1, C, W, H, D] float32 -> list of per-core input dicts."""
    W, H, D, De = cfg.W, cfg.H, cfg.D, cfg.De
    Wc, HB = cfg.Wc, cfg.HB
    x1 = np.asarray(x_1)[0].astype(ml_dtypes.bfloat16)      # [C, W, H, D]
    x2 = np.asarray(x_2)[0].astype(ml_dtypes.bfloat16)
    # padded x2: w +-1, h +-1, d in [-1, D+1)
    x2p = np.zeros((C, W + 2, H + 2, D + 2), dtype=ml_dtypes.bfloat16)
    x2p[:, 1:W + 1, 1:H + 1, 1:D + 1] = x2
    # hblk-extended h indices: row r of (h8) block = x2p h-index h8*HB + r,
    # covering h = h8*HB - 1 .. (h8+1)*HB (1-voxel halo on both sides)
    hidx = (np.arange(H8) * HB)[:, None] + np.arange(HB + 2)  # [H8, HB+2]

    consts = make_gate_consts(w0, b0, w1, b1, cfg)
    in_maps = []
    for ci in range(N_CORES):
        ws = ci * Wc
        m = dict(consts)
        m["x1"] = np.ascontiguousarray(
            _fold(x1[:, ws:ws + Wc, :, :], HB).transpose(1, 0, 2, 3))
        blk = x2p[:, ws:ws + Wc + 2, :, :]                  # [C, Wc+2, H+2, De]
        ee = blk[:, :, hidx, 1:1 + D]                       # [C, Wc+2, H8, HB+2, D]
        oo = blk[:, :, hidx, 0:De]
        m["x2e"] = np.ascontiguousarray(
            ee.transpose(3, 0, 2, 1, 4)).reshape(HB + 2, 128, Wc + 2, D)
        m["x2o"] = np.ascontiguousarray(
            oo.transpose(3, 0, 2, 1, 4)).reshape(HB + 2, 128, Wc + 2, De)
        in_maps.append(m)
    return in_maps


def assemble_output(results, cfg: Cfg):
    W, H, D = cfg.W, cfg.H, cfg.D
    Wc, HB = cfg.Wc, cfg.HB
    rows = np.empty((K, H8), dtype=np.int64)
    for k in range(K):
        for h8 in range(H8):
            rows[k, h8] = _row_of(k, h8)
    out = np.empty((W, H, D, K), dtype=np.float32)
    for ci, r in enumerate(results):
        o = np.asarray(r["out"]).reshape(HB, 216, Wc, D)
        o = o.transpose(1, 0, 2, 3)
        core = o[rows]                        # [K, H8, HB, Wc, D]
        core = core.transpose(3, 1, 2, 4, 0)  # [Wc, H8, HB, D, K]
        out[ci * Wc:(ci + 1) * Wc] = core.reshape(Wc, H, D, K)
    return out[None]


_CACHE = {}
TRACE = False           # test harness can set kernel.TRACE = True


def kernel(x_1, x_2, w0, b0, w1, b1):
    cfg = Cfg()
    if "nc" not in _CACHE:
        _CACHE["nc"] = build_nc(cfg)
    nc = _CACHE["nc"]
    in_maps = make_inputs_per_core(x_1, x_2, w0, b0, w1, b1, cfg)
    last_exc = None
    for _attempt in range(3):
        try:
            res = run_bass_kernel_spmd(nc, in_maps,
                                       core_ids=list(range(N_CORES)),
                                       trace=TRACE)
            break
        except Exception as e:  # transient NRT device errors: retry
            last_exc = e
    else:
        raise last_exc
    _CACHE["last_res"] = res
    return assemble_output(res.results, cfg)
